# revision 22
# baseline (speedup 1.0000x reference)
"""PhaseSyncAttentionV4 Trainium2 Bass kernel.

Sharding: 8 cores = B(4) x query-halves(2). Core c handles batch b=c//2,
query rows [i0, i0+512), i0 = (c%2)*512, with full keys/values over L=1024.
Everything is core-local (LayerNorm is per-row) -> no collectives.

Math notes:
  - phase_scores * geo_mag * exp(pbs) is folded into the QK matmul:
      geo = sqrt(m_i*m_j + 1e-8) ~= sqrt(m_i)*sqrt(m_j)   (abs err <= ~2e-4
      on scores; final output error ~1e-5, far below tolerance)
    so per head the score is a single contraction over 48 rows:
      32 rows of q/k  (q pre-scaled by 1/sqrt(32))
      16 rows of features F[f, t] = cs[f, t]*sqrt(bw_h[f]+1e-8)*exp(pbs_h/2)*sqrt(m_h[t])
    where cs = [cos; sin] in native [S, L] layout.
  - scores are computed transposed [j, i]: softmax sum + context are then
    plain matmuls with E=exp(scores) as the stationary operand (no
    transposes of E needed); attention_mask enters as the per-partition
    bias of the exp() activation.
  - rowsum comes from an extra ones-column appended to V (33 cols/head).
"""

import sys

sys.path.insert(0, "/opt/trn_rl_repo")

import math
import numpy as np

import concourse.bass as bass
import concourse.tile as tile
from concourse import bacc, mybir
from concourse import bass_utils
from concourse.masks import make_identity

F32 = mybir.dt.float32
F32R = mybir.dt.float32r
BF16 = mybir.dt.bfloat16
AF = mybir.ActivationFunctionType
OP = mybir.AluOpType

B, L, D = 4, 1024, 256
H, S, DH = 8, 8, 32
LQ = 512          # queries per core
NJB = L // 128    # 8 key blocks
NIB = LQ // 128   # 4 query blocks
LN_EPS = 1e-12
QSCALE = 1.0 / math.sqrt(DH)

_CACHE = {}


def _r(ap):
    return ap.bitcast(F32R)


def _build():
    """Build + compile the per-core program.

    SPMD: one program for all cores, so the query rows are ALWAYS rows
    0:512 of this core's hs. Cores handling the second query half get
    their hs/cs/mag/mask rolled by -512 along L by the host (attention
    is permutation-invariant over key positions, so rolling all L-indexed
    inputs identically is exact)."""
    i0 = 0
    nc = bacc.Bacc("TRN2", target_bir_lowering=False, debug=False,
                   enable_asserts=True, num_devices=8)

    dt_in = lambda n, s: nc.dram_tensor(n, s, F32, kind="ExternalInput").ap()
    hs_d = dt_in("hs", [L, D])
    cs_d = dt_in("cs", [2 * S, L])
    mag_d = dt_in("mag", [S, L])
    mask_d = dt_in("mask", [L])
    wq_d, wk_d, wv_d, wo_d = (dt_in(n, [D, D]) for n in ("wq", "wk", "wv", "wo"))
    bq_d, bk_d, bv_d, bo_d = (dt_in(n, [D]) for n in ("bq", "bk", "bv", "bo"))
    band_d = dt_in("band", [H, S])
    pbs_d = dt_in("pbs", [H])
    gamma_d = dt_in("gamma", [D])
    beta_d = dt_in("beta", [D])
    out_d = nc.dram_tensor("out", [LQ, D], F32, kind="ExternalOutput").ap()

    with tile.TileContext(nc) as tc:
        with (
            tc.tile_pool(name="big", bufs=1) as big,       # long-lived sbuf
            tc.tile_pool(name="epool", bufs=2) as epool,   # E tiles, per head
            tc.tile_pool(name="sm", bufs=2) as sm,         # small scratch
            tc.tile_pool(name="dram", bufs=1, space="DRAM") as dpool,
            tc.tile_pool(name="ps_sc", bufs=2, space="PSUM") as ps_sc,
            tc.tile_pool(name="ps_ct", bufs=2, space="PSUM") as ps_ct,
            tc.tile_pool(name="ps_mm", bufs=2, space="PSUM") as ps_mm,
        ):
            # ---------- stage 0: params ----------
            ident = big.tile([128, 128], F32, tag="ident")
            make_identity(nc, ident[:])

            hs_all = big.tile([128, NJB * D], F32, tag="hs")      # (jb, d)
            for jb in range(NJB):
                nc.sync.dma_start(out=hs_all[:, jb * D:(jb + 1) * D],
                                  in_=hs_d[jb * 128:(jb + 1) * 128, :])

            w_sb = {}
            for nm, wd in (("wq", wq_d), ("wk", wk_d), ("wv", wv_d)):
                traw = big.tile([128, 2 * D], F32, tag=nm + "_raw")  # (d1, dout)
                for t1 in range(2):
                    nc.sync.dma_start(out=traw[:, t1 * D:(t1 + 1) * D],
                                      in_=wd[t1 * 128:(t1 + 1) * 128, :])
                t = big.tile([128, 2 * D], F32R, tag=nm)
                nc.vector.tensor_copy(t[:], traw[:])
                w_sb[nm] = t
            wo_f = big.tile([128, 2 * D], F32, tag="wo_f")
            for t1 in range(2):
                nc.sync.dma_start(out=wo_f[:, t1 * D:(t1 + 1) * D],
                                  in_=wo_d[t1 * 128:(t1 + 1) * 128, :])
            wo_bf = big.tile([128, 2 * D], BF16, tag="wo_bf")
            nc.vector.tensor_copy(wo_bf[:], wo_f[:])

            # q/k dout is processed in 3 groups of (96, 96, 64) rows so that
            # per-head 32-row slices sit at base partitions {0, 32, 64} only
            # (offset 96 is an illegal matmul operand base on TRN2).
            GRP = [(0, 96), (96, 96), (192, 64)]
            bq_sb = big.tile([128, 3], F32, tag="bq")
            bk_sb = big.tile([128, 3], F32, tag="bk")
            for x, (o, ng) in enumerate(GRP):
                nc.sync.dma_start(out=bq_sb[0:ng, x:x + 1], in_=bq_d[o:o + ng][:, None])
                nc.sync.dma_start(out=bk_sb[0:ng, x:x + 1], in_=bk_d[o:o + ng][:, None])

            bv_b = big.tile([128, D], F32, tag="bv_b")
            nc.sync.dma_start(out=bv_b[:], in_=bass.AP(
                tensor=bv_d.tensor, offset=bv_d.offset, ap=[[0, 128], [1, D]]))
            gamma_b = big.tile([128, D], F32, tag="gamma_b")
            nc.sync.dma_start(out=gamma_b[:], in_=bass.AP(
                tensor=gamma_d.tensor, offset=gamma_d.offset, ap=[[0, 128], [1, D]]))
            beta_b = big.tile([128, D], F32, tag="beta_b")
            nc.sync.dma_start(out=beta_b[:], in_=bass.AP(
                tensor=beta_d.tensor, offset=beta_d.offset, ap=[[0, 128], [1, D]]))

            bo_f = big.tile([1, D], F32, tag="bo_f")
            nc.sync.dma_start(out=bo_f[:], in_=bo_d[None, :])
            bo_bf = big.tile([1, D], BF16, tag="bo_bf")
            nc.vector.tensor_copy(bo_bf[:], bo_f[:])
            ones_bf = big.tile([1, 128], BF16, tag="ones_bf")
            nc.vector.memset(ones_bf[:], 1.0)

            mask_sb = big.tile([128, NJB], F32, tag="mask")
            nc.sync.dma_start(out=mask_sb[:],
                              in_=mask_d.rearrange("(a p) -> p a", p=128))

            # band weights -> fscale [128,1]: row 16h+8c+t = sqrt(bw[h,t]+1e-8)*e^{pbs_h/2}
            band_sb = sm.tile([H, S], F32, tag="band")
            nc.sync.dma_start(out=band_sb[:], in_=band_d)
            pbs_sb = sm.tile([H, 1], F32, tag="pbs")
            nc.sync.dma_start(out=pbs_sb[:], in_=pbs_d[:, None])
            mx = sm.tile([H, 1], F32, tag="mx")
            nc.vector.reduce_max(out=mx[:], in_=band_sb[:], axis=mybir.AxisListType.X)
            negmx = sm.tile([H, 1], F32, tag="negmx")
            nc.vector.tensor_scalar_mul(negmx[:], mx[:], -1.0)
            be = sm.tile([H, S], F32, tag="be")
            nc.scalar.activation(out=be[:], in_=band_sb[:], func=AF.Exp, bias=negmx[:])
            bsum = sm.tile([H, 1], F32, tag="bsum")
            nc.vector.reduce_sum(out=bsum[:], in_=be[:], axis=mybir.AxisListType.X)
            brcp = sm.tile([H, 1], F32, tag="brcp")
            nc.vector.reciprocal(out=brcp[:], in_=bsum[:])
            bw = sm.tile([H, S], F32, tag="bw")
            nc.vector.tensor_scalar_mul(bw[:], be[:], brcp[:])
            eps8 = sm.tile([H, 1], F32, tag="eps8")
            nc.vector.memset(eps8[:], 1e-8)
            bwsq = sm.tile([H, S], F32, tag="bwsq")
            nc.scalar.activation(out=bwsq[:], in_=bw[:], func=AF.Sqrt, bias=eps8[:])
            es2 = sm.tile([H, 1], F32, tag="es2")
            nc.scalar.activation(out=es2[:], in_=pbs_sb[:], func=AF.Exp, scale=0.5)
            bwsq_s = sm.tile([H, S], F32, tag="bwsq_s")
            nc.vector.tensor_scalar_mul(bwsq_s[:], bwsq[:], es2[:])
            # feature scales, padded to 32 rows/head (matmul base-partition
            # alignment): tile x holds heads 3x+s at rows [32s, 32s+16),
            # rest is zero.
            NHX = [3, 3, 2]  # heads per feature/qk group
            fscales = []
            for x in range(3):
                fs = big.tile([128, 1], F32, tag=f"fscale{x}")
                nc.vector.memset(fs[:], 0.0)
                for s in range(NHX[x]):
                    for c in range(2):
                        nc.sync.dma_start(
                            out=fs[32 * s + 8 * c:32 * s + 8 * c + 8, 0:1],
                            in_=bwsq_s[3 * x + s:3 * x + s + 1, 0:S])
                fscales.append(fs)

            # ---------- stage 1: hs^T ----------
            hsT = big.tile([128, 2 * L], F32R, tag="hsT")          # (d1, j)
            for jb in range(NJB):
                for d1 in range(2):
                    tp = ps_mm.tile([128, 512], F32, tag="mm")
                    nc.tensor.transpose(tp[:, 0:128],
                                        hs_all[:, jb * D + d1 * 128:jb * D + d1 * 128 + 128],
                                        ident[:])
                    nc.vector.tensor_copy(hsT[:, d1 * L + jb * 128:d1 * L + jb * 128 + 128],
                                          tp[:, 0:128])

            # ---------- stage 2: q/k/v projections + features ----------
            qT = big.tile([128, 3 * LQ], F32R, tag="qT")           # (grp, i)
            for x, (o, ng) in enumerate(GRP):
                ps = ps_mm.tile([128, 512], F32, tag="mm")
                for d1 in range(2):
                    nc.tensor.matmul(
                        ps[0:ng, 0:LQ],
                        w_sb["wq"][:, d1 * D + o:d1 * D + o + ng],
                        hsT[:, d1 * L + i0:d1 * L + i0 + LQ],
                        start=(d1 == 0), stop=(d1 == 1))
                nc.vector.tensor_scalar(qT[0:ng, x * LQ:x * LQ + LQ], ps[0:ng, 0:LQ],
                                        bq_sb[0:ng, x:x + 1], QSCALE, OP.add, OP.mult)

            kT = big.tile([128, 3 * L], F32R, tag="kT")            # (grp, j)
            for x, (o, ng) in enumerate(GRP):
                for jh in range(2):
                    ps = ps_mm.tile([128, 512], F32, tag="mm")
                    for d1 in range(2):
                        nc.tensor.matmul(
                            ps[0:ng, 0:512],
                            w_sb["wk"][:, d1 * D + o:d1 * D + o + ng],
                            hsT[:, d1 * L + jh * 512:d1 * L + jh * 512 + 512],
                            start=(d1 == 0), stop=(d1 == 1))
                    nc.vector.tensor_scalar(
                        kT[0:ng, x * L + jh * 512:x * L + jh * 512 + 512],
                        ps[0:ng, 0:512], bk_sb[0:ng, x:x + 1], None, OP.add)

            # V augmented with a ones column: vaug[:, jb, h, 0:32]=V, [...,32]=1
            vaug = big.tile([128, NJB * H * 33], BF16, tag="vaug")
            vv = vaug[:].rearrange("p (jb h c) -> p jb h c", jb=NJB, h=H)
            nc.vector.memset(vv[:, :, :, 32:33], 1.0)
            for jb in range(NJB):
                ps = ps_mm.tile([128, 512], F32, tag="mm")
                for d1 in range(2):
                    nc.tensor.matmul(
                        ps[:, 0:D],
                        hsT[:, d1 * L + jb * 128:d1 * L + jb * 128 + 128],
                        w_sb["wv"][:, d1 * D:(d1 + 1) * D],
                        start=(d1 == 0), stop=(d1 == 1))
                nc.vector.tensor_add(vv[:, jb, :, 0:32],
                                     ps[:, 0:D].rearrange("p (h c) -> p h c", c=32),
                                     bv_b[:].rearrange("p (h c) -> p h c", c=32))

            # mag_head: [h, j] = bw @ mag ; then sqrt
            cs_sb = big.tile([2 * S, L], F32, tag="cs")
            nc.sync.dma_start(out=cs_sb[:], in_=cs_d)
            mag_sb = big.tile([S, L], F32, tag="mag")
            nc.sync.dma_start(out=mag_sb[:], in_=mag_d)
            bwT_ps = ps_mm.tile([128, 512], F32, tag="mm")
            nc.tensor.transpose(bwT_ps[0:S, 0:H], bw[:], ident[0:S, 0:H])
            bwT = sm.tile([S, H], F32R, tag="bwT")
            nc.vector.tensor_copy(bwT[:], bwT_ps[0:S, 0:H])
            mag_r = big.tile([S, L], F32R, tag="mag_r")
            nc.vector.tensor_copy(mag_r[:], mag_sb[:])
            sqrtm = big.tile([H, L], F32, tag="sqrtm")
            for jh in range(2):
                mg = ps_mm.tile([128, 512], F32, tag="mm")
                nc.tensor.matmul(mg[0:H, 0:512], bwT[:],
                                 mag_r[:, jh * 512:(jh + 1) * 512],
                                 start=True, stop=True)
                nc.scalar.activation(out=sqrtm[:, jh * 512:(jh + 1) * 512],
                                     in_=mg[0:H, 0:512], func=AF.Sqrt)

            # bounce sqrtm through DRAM so it can be partition-broadcast
            # (SBUF DMA sources cannot have 0-step partition dims)
            sq_d = dpool.tile([H, L], F32, tag="sq_d")
            nc.sync.dma_start(out=sq_d[:], in_=sqrtm[:])

            # F tiles: row 32s+f (f<16), col t = cs[f,t]*fscale*sqrtm[h,t]
            feats = []
            for x in range(3):
                fbase = big.tile([128, L], F32, tag=f"fbase{x}")
                nc.vector.memset(fbase[:], 0.0)
                smrep = big.tile([128, L], F32, tag=f"smrep{x}")
                nc.vector.memset(smrep[:], 0.0)
                for s in range(NHX[x]):
                    h = 3 * x + s
                    nc.sync.dma_start(out=fbase[32 * s:32 * s + 16, :], in_=cs_sb[:])
                    src = sq_d[h:h + 1, :]
                    nc.sync.dma_start(out=smrep[32 * s:32 * s + 16, :], in_=bass.AP(
                        tensor=src.tensor, offset=src.offset,
                        ap=[[0, 16]] + src.ap[1:]))
                feat = big.tile([128, L], F32R, tag=f"feat{x}")
                nc.vector.scalar_tensor_tensor(out=feat[:], in0=fbase[:],
                                               scalar=fscales[x][:], in1=smrep[:],
                                               op0=OP.mult, op1=OP.mult)
                feats.append(feat)

            # ---------- stage 3: attention (software-pipelined over heads) ----------
            ctx_all = big.tile([128, NIB * D], F32, tag="ctx")    # (ib, h, c)
            rcp_sb = big.tile([128, H * NIB], F32, tag="rcp")     # (h, ib)

            e_tiles = [None] * H

            def emit_scores(h):
                x, hr = h // 3, (h % 3) * 32
                eh = epool.tile([128, NJB * 512], BF16, tag="E")
                e_tiles[h] = eh
                for jp in range(NJB // 2):
                    sc = ps_sc.tile([128, 1024], F32, tag="sc")
                    for q in range(2):
                        jb = 2 * jp + q
                        col = q * 512
                        nc.tensor.matmul(
                            sc[:, col:col + 512],
                            kT[hr:hr + 32, x * L + jb * 128:x * L + jb * 128 + 128],
                            qT[hr:hr + 32, x * LQ:x * LQ + LQ],
                            start=True, stop=False)
                        ft = feats[x]
                        nc.tensor.matmul(
                            sc[:, col:col + 512],
                            ft[hr:hr + 32, jb * 128:jb * 128 + 128],
                            ft[hr:hr + 32, i0:i0 + LQ],
                            start=False, stop=True)
                    for q in range(2):
                        jb = 2 * jp + q
                        nc.scalar.activation(
                            out=eh[:, jb * 512:(jb + 1) * 512],
                            in_=sc[:, q * 512:(q + 1) * 512],
                            func=AF.Exp, bias=mask_sb[:, jb:jb + 1])

            def emit_ctx(h):
                eh = e_tiles[h]
                ct = ps_ct.tile([128, NIB * 33], F32, tag="ct")
                for ib in range(NIB):
                    for jb in range(NJB):
                        nc.tensor.matmul(
                            ct[:, ib * 33:ib * 33 + 33],
                            eh[:, jb * 512 + ib * 128:jb * 512 + ib * 128 + 128],
                            vaug[:, (jb * H + h) * 33:(jb * H + h) * 33 + 33],
                            start=(jb == 0), stop=(jb == NJB - 1))
                ctr = ct[:].rearrange("p (ib c) -> p ib c", c=33)
                nc.vector.reciprocal(out=rcp_sb[:, h * NIB:(h + 1) * NIB],
                                     in_=ctr[:, :, 32:33])
                rc = rcp_sb[:, h * NIB:(h + 1) * NIB]
                rcb = bass.AP(tensor=rc.tensor, offset=rc.offset,
                              ap=[rc.ap[0], [rc.ap[1][0], NIB], [0, 32]])
                dst = ctx_all[:].rearrange("p (ib hh c) -> p ib hh c", ib=NIB, hh=H)
                nc.vector.tensor_tensor(out=dst[:, :, h, :], in0=ctr[:, :, 0:32],
                                        in1=rcb, op=OP.mult)

            for h in range(H):
                emit_scores(h)
                if h > 0:
                    emit_ctx(h - 1)
            emit_ctx(H - 1)

            # ---------- stage 4: out-proj + residual + LayerNorm ----------
            epsln = big.tile([128, 1], F32, tag="epsln")
            nc.vector.memset(epsln[:], LN_EPS)
            for ib in range(NIB):
                ctxT = sm.tile([128, 256], BF16, tag="ctxT")
                for t in range(2):
                    tp = ps_mm.tile([128, 512], F32, tag="mm")
                    nc.tensor.transpose(tp[:, 0:128],
                                        ctx_all[:, ib * D + t * 128:ib * D + t * 128 + 128],
                                        ident[:])
                    nc.vector.tensor_copy(ctxT[:, t * 128:(t + 1) * 128], tp[:, 0:128])
                op_ps = ps_mm.tile([128, 512], F32, tag="mm")
                for t in range(2):
                    nc.tensor.matmul(op_ps[:, 0:D], ctxT[:, t * 128:(t + 1) * 128],
                                     wo_bf[:, t * D:(t + 1) * D],
                                     start=(t == 0), stop=False)
                nc.tensor.matmul(op_ps[:, 0:D], ones_bf[:], bo_bf[:],
                                 start=False, stop=True)

                x = sm.tile([128, D], F32, tag="x")
                ihb = i0 // 128 + ib
                nc.vector.tensor_add(x[:], op_ps[:, 0:D],
                                     hs_all[:, ihb * D:(ihb + 1) * D])
                stats = sm.tile([128, 6], F32, tag="stats")
                nc.vector.bn_stats(out=stats[:], in_=x[:])
                mv = sm.tile([128, 2], F32, tag="mv")
                nc.vector.bn_aggr(out=mv[:], in_=stats[:])
                std = sm.tile([128, 1], F32, tag="std")
                nc.scalar.activation(out=std[:], in_=mv[:, 1:2], func=AF.Sqrt,
                                     bias=epsln[:])
                rstd = sm.tile([128, 1], F32, tag="rstd")
                nc.vector.reciprocal(out=rstd[:], in_=std[:])
                xc = sm.tile([128, D], F32, tag="xc")
                nc.vector.tensor_scalar(xc[:], x[:], mv[:, 0:1], rstd[:],
                                        OP.subtract, OP.mult)
                y = sm.tile([128, D], F32, tag="y")
                nc.vector.tensor_mul(y[:], xc[:], gamma_b[:])
                y2 = sm.tile([128, D], F32, tag="y2")
                nc.vector.tensor_add(y2[:], y[:], beta_b[:])
                nc.sync.dma_start(out=out_d[ib * 128:(ib + 1) * 128, :], in_=y2[:])

    nc.compile()
    return nc


def kernel(**inputs):
    hs = np.asarray(inputs["hidden_states"], np.float32)
    am = np.asarray(inputs["attention_mask"], np.float32)
    cos = np.asarray(inputs["cos_phi"], np.float32)
    sin = np.asarray(inputs["sin_phi"], np.float32)
    mag = np.asarray(inputs["mag"], np.float32)

    if "nc" not in _CACHE:
        _CACHE["nc"] = _build()
    nc = _CACHE["nc"]

    in_maps = []
    for c in range(8):
        b, half = divmod(c, 2)
        r = half * LQ  # roll amount: odd cores see L rotated by -512

        def roll(x, axis):
            return np.roll(x, -r, axis=axis) if r else x

        m = {
            "hs": roll(hs[b], 0),
            "cs": roll(np.concatenate([cos[b], sin[b]], axis=0), 1),
            "mag": roll(mag[b], 1),
            "mask": roll(np.ascontiguousarray(
                np.broadcast_to(am[b, 0, 0], (L,))), 0),
            "wq": np.asarray(inputs["Wq"], np.float32),
            "wk": np.asarray(inputs["Wk"], np.float32),
            "wv": np.asarray(inputs["Wv"], np.float32),
            "wo": np.asarray(inputs["Wo"], np.float32),
            "bq": np.asarray(inputs["bq"], np.float32),
            "bk": np.asarray(inputs["bk"], np.float32),
            "bv": np.asarray(inputs["bv"], np.float32),
            "bo": np.asarray(inputs["bo"], np.float32),
            "band": np.asarray(inputs["band_logits"], np.float32),
            "pbs": np.asarray(inputs["phase_bias_scale"], np.float32),
            "gamma": np.asarray(inputs["ln_gamma"], np.float32),
            "beta": np.asarray(inputs["ln_beta"], np.float32),
        }
        in_maps.append(m)

    _CACHE["last_in_maps"] = in_maps
    globals()["_LAST_IN_MAPS"] = in_maps
    res = bass_utils.run_bass_kernel_spmd(nc, in_maps, core_ids=list(range(8)))
    out = np.empty((B, L, D), np.float32)
    for c in range(8):
        b, half = divmod(c, 2)
        out[b, half * LQ:(half + 1) * LQ, :] = res.results[c]["out"]
    return out


# revision 26
# speedup vs baseline: 1.0345x; 1.0345x over previous
"""PhaseSyncAttentionV4 Trainium2 Bass kernel.

Sharding: 8 cores = B(4) x query-halves(2). Core c handles batch b=c//2,
query rows [i0, i0+512), i0 = (c%2)*512, with full keys/values over L=1024.
Everything is core-local (LayerNorm is per-row) -> no collectives.

Math notes:
  - phase_scores * geo_mag * exp(pbs) is folded into the QK matmul:
      geo = sqrt(m_i*m_j + 1e-8) ~= sqrt(m_i)*sqrt(m_j)   (abs err <= ~2e-4
      on scores; final output error ~1e-5, far below tolerance)
    so per head the score is a single contraction over 48 rows:
      32 rows of q/k  (q pre-scaled by 1/sqrt(32))
      16 rows of features F[f, t] = cs[f, t]*sqrt(bw_h[f]+1e-8)*exp(pbs_h/2)*sqrt(m_h[t])
    where cs = [cos; sin] in native [S, L] layout.
  - scores are computed transposed [j, i]: softmax sum + context are then
    plain matmuls with E=exp(scores) as the stationary operand (no
    transposes of E needed); attention_mask enters as the per-partition
    bias of the exp() activation.
  - rowsum comes from an extra ones-column appended to V (33 cols/head).
"""

import sys

sys.path.insert(0, "/opt/trn_rl_repo")

import math
import numpy as np

import concourse.bass as bass
import concourse.tile as tile
from concourse import bacc, mybir
from concourse import bass_utils
from concourse.masks import make_identity

F32 = mybir.dt.float32
F32R = mybir.dt.float32r
BF16 = mybir.dt.bfloat16
AF = mybir.ActivationFunctionType
OP = mybir.AluOpType

B, L, D = 4, 1024, 256
H, S, DH = 8, 8, 32
LQ = 512          # queries per core
NJB = L // 128    # 8 key blocks
NIB = LQ // 128   # 4 query blocks
LN_EPS = 1e-12
QSCALE = 1.0 / math.sqrt(DH)

_CACHE = {}


def _r(ap):
    return ap.bitcast(F32R)


def _build():
    """Build + compile the per-core program.

    SPMD: one program for all cores, so the query rows are ALWAYS rows
    0:512 of this core's hs. Cores handling the second query half get
    their hs/cs/mag/mask rolled by -512 along L by the host (attention
    is permutation-invariant over key positions, so rolling all L-indexed
    inputs identically is exact)."""
    i0 = 0
    nc = bacc.Bacc("TRN2", target_bir_lowering=False, debug=False,
                   enable_asserts=True, num_devices=8)

    dt_in = lambda n, s: nc.dram_tensor(n, s, F32, kind="ExternalInput").ap()
    hs_d = dt_in("hs", [L, D])
    cs_d = dt_in("cs", [2 * S, L])
    mag_d = dt_in("mag", [S, L])
    mask_d = dt_in("mask", [L])
    wq_d, wk_d, wv_d, wo_d = (dt_in(n, [D, D]) for n in ("wq", "wk", "wv", "wo"))
    bq_d, bk_d, bv_d, bo_d = (dt_in(n, [D]) for n in ("bq", "bk", "bv", "bo"))
    band_d = dt_in("band", [H, S])
    pbs_d = dt_in("pbs", [H])
    gamma_d = dt_in("gamma", [D])
    beta_d = dt_in("beta", [D])
    out_d = nc.dram_tensor("out", [LQ, D], F32, kind="ExternalOutput").ap()

    with tile.TileContext(nc) as tc:
        with (
            tc.tile_pool(name="big", bufs=1) as big,       # long-lived sbuf
            tc.tile_pool(name="epool", bufs=2) as epool,   # E tiles, per head
            tc.tile_pool(name="sm", bufs=2) as sm,         # small scratch
            tc.tile_pool(name="dram", bufs=1, space="DRAM") as dpool,
            tc.tile_pool(name="ps_sc", bufs=2, space="PSUM") as ps_sc,
            tc.tile_pool(name="ps_ct", bufs=2, space="PSUM") as ps_ct,
            tc.tile_pool(name="ps_mm", bufs=2, space="PSUM") as ps_mm,
        ):
            # ---------- stage 0: params ----------
            ident = big.tile([128, 128], F32, tag="ident")
            make_identity(nc, ident[:])

            hs_all = big.tile([128, NJB * D], F32, tag="hs")      # (jb, d)
            for jb in range(NJB):
                nc.sync.dma_start(out=hs_all[:, jb * D:(jb + 1) * D],
                                  in_=hs_d[jb * 128:(jb + 1) * 128, :])

            w_sb = {}
            for nm, wd in (("wq", wq_d), ("wk", wk_d), ("wv", wv_d)):
                traw = big.tile([128, 2 * D], F32, tag=nm + "_raw")  # (d1, dout)
                for t1 in range(2):
                    nc.sync.dma_start(out=traw[:, t1 * D:(t1 + 1) * D],
                                      in_=wd[t1 * 128:(t1 + 1) * 128, :])
                t = big.tile([128, 2 * D], BF16, tag=nm)
                nc.vector.tensor_copy(t[:], traw[:])
                w_sb[nm] = t
            wo_f = big.tile([128, 2 * D], F32, tag="wo_f")
            for t1 in range(2):
                nc.sync.dma_start(out=wo_f[:, t1 * D:(t1 + 1) * D],
                                  in_=wo_d[t1 * 128:(t1 + 1) * 128, :])
            wo_bf = big.tile([128, 2 * D], BF16, tag="wo_bf")
            nc.vector.tensor_copy(wo_bf[:], wo_f[:])

            # q/k dout is processed in 3 groups of (96, 96, 64) rows so that
            # per-head 32-row slices sit at base partitions {0, 32, 64} only
            # (offset 96 is an illegal matmul operand base on TRN2).
            GRP = [(0, 96), (96, 96), (192, 64)]
            bq_sb = big.tile([128, 3], F32, tag="bq")
            bk_sb = big.tile([128, 3], F32, tag="bk")
            for x, (o, ng) in enumerate(GRP):
                nc.sync.dma_start(out=bq_sb[0:ng, x:x + 1], in_=bq_d[o:o + ng][:, None])
                nc.sync.dma_start(out=bk_sb[0:ng, x:x + 1], in_=bk_d[o:o + ng][:, None])

            bv_b = big.tile([128, D], F32, tag="bv_b")
            nc.sync.dma_start(out=bv_b[:], in_=bass.AP(
                tensor=bv_d.tensor, offset=bv_d.offset, ap=[[0, 128], [1, D]]))
            gamma_b = big.tile([128, D], F32, tag="gamma_b")
            nc.sync.dma_start(out=gamma_b[:], in_=bass.AP(
                tensor=gamma_d.tensor, offset=gamma_d.offset, ap=[[0, 128], [1, D]]))
            beta_b = big.tile([128, D], F32, tag="beta_b")
            nc.sync.dma_start(out=beta_b[:], in_=bass.AP(
                tensor=beta_d.tensor, offset=beta_d.offset, ap=[[0, 128], [1, D]]))

            bo_f = big.tile([1, D], F32, tag="bo_f")
            nc.sync.dma_start(out=bo_f[:], in_=bo_d[None, :])
            bo_bf = big.tile([1, D], BF16, tag="bo_bf")
            nc.vector.tensor_copy(bo_bf[:], bo_f[:])
            ones_bf = big.tile([1, 128], BF16, tag="ones_bf")
            nc.vector.memset(ones_bf[:], 1.0)
            ones_bfL = big.tile([1, L], BF16, tag="ones_bfL")
            nc.vector.memset(ones_bfL[:], 1.0)

            mask_sb = big.tile([128, NJB], F32, tag="mask")
            nc.sync.dma_start(out=mask_sb[:],
                              in_=mask_d.rearrange("(a p) -> p a", p=128))

            # band weights -> fscale [128,1]: row 16h+8c+t = sqrt(bw[h,t]+1e-8)*e^{pbs_h/2}
            band_sb = sm.tile([H, S], F32, tag="band")
            nc.sync.dma_start(out=band_sb[:], in_=band_d)
            pbs_sb = sm.tile([H, 1], F32, tag="pbs")
            nc.sync.dma_start(out=pbs_sb[:], in_=pbs_d[:, None])
            mx = sm.tile([H, 1], F32, tag="mx")
            nc.vector.reduce_max(out=mx[:], in_=band_sb[:], axis=mybir.AxisListType.X)
            negmx = sm.tile([H, 1], F32, tag="negmx")
            nc.vector.tensor_scalar_mul(negmx[:], mx[:], -1.0)
            be = sm.tile([H, S], F32, tag="be")
            nc.scalar.activation(out=be[:], in_=band_sb[:], func=AF.Exp, bias=negmx[:])
            bsum = sm.tile([H, 1], F32, tag="bsum")
            nc.vector.reduce_sum(out=bsum[:], in_=be[:], axis=mybir.AxisListType.X)
            brcp = sm.tile([H, 1], F32, tag="brcp")
            nc.vector.reciprocal(out=brcp[:], in_=bsum[:])
            bw = sm.tile([H, S], F32, tag="bw")
            nc.vector.tensor_scalar_mul(bw[:], be[:], brcp[:])
            eps8 = sm.tile([H, 1], F32, tag="eps8")
            nc.vector.memset(eps8[:], 1e-8)
            bwsq = sm.tile([H, S], F32, tag="bwsq")
            nc.scalar.activation(out=bwsq[:], in_=bw[:], func=AF.Sqrt, bias=eps8[:])
            es2 = sm.tile([H, 1], F32, tag="es2")
            nc.scalar.activation(out=es2[:], in_=pbs_sb[:], func=AF.Exp, scale=0.5)
            bwsq_s = sm.tile([H, S], F32, tag="bwsq_s")
            nc.vector.tensor_scalar_mul(bwsq_s[:], bwsq[:], es2[:])
            # feature scales, padded to 32 rows/head (matmul base-partition
            # alignment): tile x holds heads 3x+s at rows [32s, 32s+16),
            # rest is zero.
            NHX = [3, 3, 2]  # heads per feature/qk group
            fscales = []
            for x in range(3):
                fs = big.tile([128, 1], F32, tag=f"fscale{x}")
                nc.vector.memset(fs[:], 0.0)
                for s in range(NHX[x]):
                    for c in range(2):
                        nc.sync.dma_start(
                            out=fs[32 * s + 8 * c:32 * s + 8 * c + 8, 0:1],
                            in_=bwsq_s[3 * x + s:3 * x + s + 1, 0:S])
                fscales.append(fs)

            # ---------- stage 1: hs^T ----------
            hsT = big.tile([128, 2 * L], BF16, tag="hsT")          # (d1, j)
            for jb in range(NJB):
                for d1 in range(2):
                    tp = ps_mm.tile([128, 512], F32, tag="mm")
                    nc.tensor.transpose(tp[:, 0:128],
                                        hs_all[:, jb * D + d1 * 128:jb * D + d1 * 128 + 128],
                                        ident[:])
                    nc.vector.tensor_copy(hsT[:, d1 * L + jb * 128:d1 * L + jb * 128 + 128],
                                          tp[:, 0:128])

            # ---------- stage 2: q/k/v projections + features ----------
            qT = big.tile([128, 3 * LQ], BF16, tag="qT")           # (grp, i)
            for x, (o, ng) in enumerate(GRP):
                ps = ps_mm.tile([128, 512], F32, tag="mm")
                for d1 in range(2):
                    nc.tensor.matmul(
                        ps[0:ng, 0:LQ],
                        w_sb["wq"][:, d1 * D + o:d1 * D + o + ng],
                        hsT[:, d1 * L + i0:d1 * L + i0 + LQ],
                        start=(d1 == 0), stop=(d1 == 1))
                nc.vector.tensor_scalar(qT[0:ng, x * LQ:x * LQ + LQ], ps[0:ng, 0:LQ],
                                        bq_sb[0:ng, x:x + 1], QSCALE, OP.add, OP.mult)

            kT = big.tile([128, 3 * L], BF16, tag="kT")            # (grp, j)
            for x, (o, ng) in enumerate(GRP):
                for jh in range(2):
                    ps = ps_mm.tile([128, 512], F32, tag="mm")
                    for d1 in range(2):
                        nc.tensor.matmul(
                            ps[0:ng, 0:512],
                            w_sb["wk"][:, d1 * D + o:d1 * D + o + ng],
                            hsT[:, d1 * L + jh * 512:d1 * L + jh * 512 + 512],
                            start=(d1 == 0), stop=(d1 == 1))
                    nc.vector.tensor_scalar(
                        kT[0:ng, x * L + jh * 512:x * L + jh * 512 + 512],
                        ps[0:ng, 0:512], bk_sb[0:ng, x:x + 1], None, OP.add)

            # V augmented with a ones column: vaug[:, jb, h, 0:32]=V, [...,32]=1
            vaug = big.tile([128, NJB * H * 33], BF16, tag="vaug")
            vv = vaug[:].rearrange("p (jb h c) -> p jb h c", jb=NJB, h=H)
            nc.vector.memset(vv[:, :, :, 32:33], 1.0)
            for jb in range(NJB):
                ps = ps_mm.tile([128, 512], F32, tag="mm")
                for d1 in range(2):
                    nc.tensor.matmul(
                        ps[:, 0:D],
                        hsT[:, d1 * L + jb * 128:d1 * L + jb * 128 + 128],
                        w_sb["wv"][:, d1 * D:(d1 + 1) * D],
                        start=(d1 == 0), stop=(d1 == 1))
                nc.vector.tensor_add(vv[:, jb, :, 0:32],
                                     ps[:, 0:D].rearrange("p (h c) -> p h c", c=32),
                                     bv_b[:].rearrange("p (h c) -> p h c", c=32))

            # mag_head: [h, j] = bw @ mag ; then sqrt
            cs_sb = big.tile([2 * S, L], F32, tag="cs")
            nc.sync.dma_start(out=cs_sb[:], in_=cs_d)
            mag_sb = big.tile([S, L], F32, tag="mag")
            nc.sync.dma_start(out=mag_sb[:], in_=mag_d)
            bwT_ps = ps_mm.tile([128, 512], F32, tag="mm")
            nc.tensor.transpose(bwT_ps[0:S, 0:H], bw[:], ident[0:S, 0:H])
            bwT = sm.tile([S, H], BF16, tag="bwT")
            nc.vector.tensor_copy(bwT[:], bwT_ps[0:S, 0:H])
            mag_r = big.tile([S, L], BF16, tag="mag_r")
            nc.vector.tensor_copy(mag_r[:], mag_sb[:])
            sqrtm = big.tile([H, L], F32, tag="sqrtm")
            for jh in range(2):
                mg = ps_mm.tile([128, 512], F32, tag="mm")
                nc.tensor.matmul(mg[0:H, 0:512], bwT[:],
                                 mag_r[:, jh * 512:(jh + 1) * 512],
                                 start=True, stop=True)
                nc.scalar.activation(out=sqrtm[:, jh * 512:(jh + 1) * 512],
                                     in_=mg[0:H, 0:512], func=AF.Sqrt)

            # bounce sqrtm through DRAM so it can be partition-broadcast
            # (SBUF DMA sources cannot have 0-step partition dims)
            sq_d = dpool.tile([H, L], F32, tag="sq_d")
            nc.sync.dma_start(out=sq_d[:], in_=sqrtm[:])

            # F tiles: row 32s+f (f<16), col t = cs[f,t]*fscale*sqrtm[h,t].
            # featK additionally carries mask[j] in row 32s+16; featQ carries
            # 1.0 there, so the scores matmul contributes mask[j]*1 -- the
            # attention mask folds into the matmul and exp() needs no bias.
            featKs, featQs = [], []
            for x in range(3):
                fbase = big.tile([128, L], F32, tag=f"fbase{x}")
                nc.vector.memset(fbase[:], 0.0)
                smrep = big.tile([128, L], F32, tag=f"smrep{x}")
                nc.vector.memset(smrep[:], 0.0)
                for s in range(NHX[x]):
                    h = 3 * x + s
                    nc.sync.dma_start(out=fbase[32 * s:32 * s + 16, :], in_=cs_sb[:])
                    src = sq_d[h:h + 1, :]
                    nc.sync.dma_start(out=smrep[32 * s:32 * s + 16, :], in_=bass.AP(
                        tensor=src.tensor, offset=src.offset,
                        ap=[[0, 16]] + src.ap[1:]))
                featK = big.tile([128, L], BF16, tag=f"featK{x}")
                nc.vector.scalar_tensor_tensor(out=featK[:], in0=fbase[:],
                                               scalar=fscales[x][:], in1=smrep[:],
                                               op0=OP.mult, op1=OP.mult)
                featQ = big.tile([128, L], BF16, tag=f"featQ{x}")
                nc.vector.tensor_copy(featQ[:], featK[:])
                for s in range(NHX[x]):
                    nc.gpsimd.dma_start(out=featK[32 * s + 16:32 * s + 17, :],
                                        in_=mask_d[None, :])
                    nc.sync.dma_start(out=featQ[32 * s + 16:32 * s + 17, :],
                                      in_=ones_bfL[:])
                featKs.append(featK)
                featQs.append(featQ)

            # ---------- stage 3: attention (software-pipelined over heads) ----------
            ctx_all = big.tile([128, NIB * D], F32, tag="ctx")    # (ib, h, c)
            rcp_sb = big.tile([128, H * NIB], F32, tag="rcp")     # (h, ib)

            e_tiles = [None] * H

            def emit_scores(h):
                x, hr = h // 3, (h % 3) * 32
                eh = epool.tile([128, NJB * 512], BF16, tag="E")
                e_tiles[h] = eh
                for jp in range(NJB // 2):
                    sc = ps_sc.tile([128, 1024], F32, tag="sc")
                    for q in range(2):
                        jb = 2 * jp + q
                        col = q * 512
                        nc.tensor.matmul(
                            sc[:, col:col + 512],
                            kT[hr:hr + 32, x * L + jb * 128:x * L + jb * 128 + 128],
                            qT[hr:hr + 32, x * LQ:x * LQ + LQ],
                            start=True, stop=False)
                        nc.tensor.matmul(
                            sc[:, col:col + 512],
                            featKs[x][hr:hr + 32, jb * 128:jb * 128 + 128],
                            featQs[x][hr:hr + 32, i0:i0 + LQ],
                            start=False, stop=True)
                    nc.scalar.activation(
                        out=eh[:, jp * 1024:(jp + 1) * 1024],
                        in_=sc[:, 0:1024], func=AF.Exp)

            def emit_ctx(h):
                eh = e_tiles[h]
                ct = ps_ct.tile([128, NIB * 33], F32, tag="ct")
                for ib in range(NIB):
                    for jb in range(NJB):
                        nc.tensor.matmul(
                            ct[:, ib * 33:ib * 33 + 33],
                            eh[:, jb * 512 + ib * 128:jb * 512 + ib * 128 + 128],
                            vaug[:, (jb * H + h) * 33:(jb * H + h) * 33 + 33],
                            start=(jb == 0), stop=(jb == NJB - 1))
                ctr = ct[:].rearrange("p (ib c) -> p ib c", c=33)
                nc.vector.reciprocal(out=rcp_sb[:, h * NIB:(h + 1) * NIB],
                                     in_=ctr[:, :, 32:33])
                rc = rcp_sb[:, h * NIB:(h + 1) * NIB]
                rcb = bass.AP(tensor=rc.tensor, offset=rc.offset,
                              ap=[rc.ap[0], [rc.ap[1][0], NIB], [0, 32]])
                dst = ctx_all[:].rearrange("p (ib hh c) -> p ib hh c", ib=NIB, hh=H)
                nc.vector.tensor_tensor(out=dst[:, :, h, :], in0=ctr[:, :, 0:32],
                                        in1=rcb, op=OP.mult)

            for h in range(H):
                emit_scores(h)
                if h > 0:
                    emit_ctx(h - 1)
            emit_ctx(H - 1)

            # ---------- stage 4: out-proj + residual + LayerNorm ----------
            epsln = big.tile([128, 1], F32, tag="epsln")
            nc.vector.memset(epsln[:], LN_EPS)
            for ib in range(NIB):
                ctxT = sm.tile([128, 256], BF16, tag="ctxT")
                for t in range(2):
                    tp = ps_mm.tile([128, 512], F32, tag="mm")
                    nc.tensor.transpose(tp[:, 0:128],
                                        ctx_all[:, ib * D + t * 128:ib * D + t * 128 + 128],
                                        ident[:])
                    nc.vector.tensor_copy(ctxT[:, t * 128:(t + 1) * 128], tp[:, 0:128])
                op_ps = ps_mm.tile([128, 512], F32, tag="mm")
                for t in range(2):
                    nc.tensor.matmul(op_ps[:, 0:D], ctxT[:, t * 128:(t + 1) * 128],
                                     wo_bf[:, t * D:(t + 1) * D],
                                     start=(t == 0), stop=False)
                nc.tensor.matmul(op_ps[:, 0:D], ones_bf[:], bo_bf[:],
                                 start=False, stop=True)

                x = sm.tile([128, D], F32, tag="x")
                ihb = i0 // 128 + ib
                nc.vector.tensor_add(x[:], op_ps[:, 0:D],
                                     hs_all[:, ihb * D:(ihb + 1) * D])
                stats = sm.tile([128, 6], F32, tag="stats")
                nc.vector.bn_stats(out=stats[:], in_=x[:])
                mv = sm.tile([128, 2], F32, tag="mv")
                nc.vector.bn_aggr(out=mv[:], in_=stats[:])
                std = sm.tile([128, 1], F32, tag="std")
                nc.scalar.activation(out=std[:], in_=mv[:, 1:2], func=AF.Sqrt,
                                     bias=epsln[:])
                rstd = sm.tile([128, 1], F32, tag="rstd")
                nc.vector.reciprocal(out=rstd[:], in_=std[:])
                xc = sm.tile([128, D], F32, tag="xc")
                nc.vector.tensor_scalar(xc[:], x[:], mv[:, 0:1], rstd[:],
                                        OP.subtract, OP.mult)
                y = sm.tile([128, D], F32, tag="y")
                nc.vector.tensor_mul(y[:], xc[:], gamma_b[:])
                y2 = sm.tile([128, D], F32, tag="y2")
                nc.vector.tensor_add(y2[:], y[:], beta_b[:])
                nc.sync.dma_start(out=out_d[ib * 128:(ib + 1) * 128, :], in_=y2[:])

    nc.compile()
    return nc


def kernel(**inputs):
    hs = np.asarray(inputs["hidden_states"], np.float32)
    am = np.asarray(inputs["attention_mask"], np.float32)
    cos = np.asarray(inputs["cos_phi"], np.float32)
    sin = np.asarray(inputs["sin_phi"], np.float32)
    mag = np.asarray(inputs["mag"], np.float32)

    if "nc" not in _CACHE:
        _CACHE["nc"] = _build()
    nc = _CACHE["nc"]

    in_maps = []
    for c in range(8):
        b, half = divmod(c, 2)
        r = half * LQ  # roll amount: odd cores see L rotated by -512

        def roll(x, axis):
            return np.roll(x, -r, axis=axis) if r else x

        m = {
            "hs": roll(hs[b], 0),
            "cs": roll(np.concatenate([cos[b], sin[b]], axis=0), 1),
            "mag": roll(mag[b], 1),
            "mask": roll(np.ascontiguousarray(
                np.broadcast_to(am[b, 0, 0], (L,))), 0),
            "wq": np.asarray(inputs["Wq"], np.float32),
            "wk": np.asarray(inputs["Wk"], np.float32),
            "wv": np.asarray(inputs["Wv"], np.float32),
            "wo": np.asarray(inputs["Wo"], np.float32),
            "bq": np.asarray(inputs["bq"], np.float32),
            "bk": np.asarray(inputs["bk"], np.float32),
            "bv": np.asarray(inputs["bv"], np.float32),
            "bo": np.asarray(inputs["bo"], np.float32),
            "band": np.asarray(inputs["band_logits"], np.float32),
            "pbs": np.asarray(inputs["phase_bias_scale"], np.float32),
            "gamma": np.asarray(inputs["ln_gamma"], np.float32),
            "beta": np.asarray(inputs["ln_beta"], np.float32),
        }
        in_maps.append(m)

    _CACHE["last_in_maps"] = in_maps
    globals()["_LAST_IN_MAPS"] = in_maps
    res = bass_utils.run_bass_kernel_spmd(nc, in_maps, core_ids=list(range(8)))
    out = np.empty((B, L, D), np.float32)
    for c in range(8):
        b, half = divmod(c, 2)
        out[b, half * LQ:(half + 1) * LQ, :] = res.results[c]["out"]
    return out


# revision 32
# speedup vs baseline: 1.1954x; 1.1555x over previous
"""PhaseSyncAttentionV4 Trainium2 Bass kernel.

Sharding: 8 cores = B(4) x query-halves(2). Core c handles batch b=c//2,
query rows [i0, i0+512), with full keys/values over L=1024. Everything is
core-local (LayerNorm is per-row) -> no collectives. SPMD: one program for
all cores; odd cores get their hs/cs/mag/mask rolled by -512 along L by
the host (attention is permutation-invariant over key positions), so the
query rows are always rows 0:512.

Math notes:
  - phase_scores * geo_mag * exp(pbs) folds into the QK matmul via
      geo = sqrt(m_i*m_j + 1e-8) ~= sqrt(m_i)*sqrt(m_j)
    (final output error ~1e-5, far below tolerance), giving per head a
    single score matmul with K=49 contraction rows:
      32 rows q/k (q pre-scaled by 1/sqrt(32) with bias)
      16 rows features F[f,t] = cs[f,t]*sqrt(bw_h[f]+1e-8)*e^{pbs_h/2}*sqrt(m_h[t])
       1 row  mask: KF carries mask[j], QF carries 1.0 -> contributes mask[j]
    where cs = [cos; sin] is the input's native [S, L] layout.
  - scores are computed transposed [j, i]: exp(E) is then used directly as
    the stationary matmul operand for both context and softmax row-sums
    (ones column appended to V), so no transposes of E are ever needed.
"""

import sys

sys.path.insert(0, "/opt/trn_rl_repo")

import math
import numpy as np

import concourse.bass as bass
import concourse.tile as tile
from concourse import bacc, mybir
from concourse import bass_utils
from concourse.masks import make_identity

F32 = mybir.dt.float32
BF16 = mybir.dt.bfloat16
AF = mybir.ActivationFunctionType
OP = mybir.AluOpType

B, L, D = 4, 1024, 256
H, S, DH = 8, 8, 32
LQ = 512          # queries per core
NJB = L // 128    # 8 key blocks
NIB = LQ // 128   # 4 query blocks
LN_EPS = 1e-12
QSCALE = 1.0 / math.sqrt(DH)
NHX = [3, 3, 2]   # heads per 96-row feature group

_CACHE = {}


def _build():
    nc = bacc.Bacc("TRN2", target_bir_lowering=False, debug=False,
                   enable_asserts=True, num_devices=8)

    dt_in = lambda n, s: nc.dram_tensor(n, s, F32, kind="ExternalInput").ap()
    hs_d = dt_in("hs", [L, D])
    cs_d = dt_in("cs", [2 * S, L])
    mag_d = dt_in("mag", [S, L])
    mask_d = dt_in("mask", [L])
    wq_d, wk_d, wv_d, wo_d = (dt_in(n, [D, D]) for n in ("wq", "wk", "wv", "wo"))
    bq_d, bk_d, bv_d, bo_d = (dt_in(n, [D]) for n in ("bq", "bk", "bv", "bo"))
    band_d = dt_in("band", [H, S])
    pbs_d = dt_in("pbs", [H])
    gamma_d = dt_in("gamma", [D])
    beta_d = dt_in("beta", [D])
    out_d = nc.dram_tensor("out", [LQ, D], F32, kind="ExternalOutput").ap()

    with tile.TileContext(nc) as tc:
        with (
            tc.tile_pool(name="big", bufs=1) as big,
            tc.tile_pool(name="epool", bufs=2) as epool,
            tc.tile_pool(name="kf", bufs=3) as kfp,
            tc.tile_pool(name="sm", bufs=2) as sm,
            tc.tile_pool(name="dram", bufs=1, space="DRAM") as dpool,
            tc.tile_pool(name="ps_sc", bufs=2, space="PSUM") as ps_sc,
            tc.tile_pool(name="ps_ct", bufs=2, space="PSUM") as ps_ct,
            tc.tile_pool(name="ps_mm", bufs=2, space="PSUM") as ps_mm,
        ):
            ident = big.tile([128, 128], F32, tag="ident")
            make_identity(nc, ident[:])

            # ============ feature chain (long latency; start first) ========
            # band softmax -> bw [H, S]
            band_sb = sm.tile([H, S], F32, tag="band")
            nc.gpsimd.dma_start(out=band_sb[:], in_=band_d)
            pbs_sb = sm.tile([H, 1], F32, tag="pbs")
            nc.gpsimd.dma_start(out=pbs_sb[:], in_=pbs_d[:, None])
            mx = sm.tile([H, 1], F32, tag="mx")
            nc.vector.reduce_max(out=mx[:], in_=band_sb[:], axis=mybir.AxisListType.X)
            negmx = sm.tile([H, 1], F32, tag="negmx")
            nc.vector.tensor_scalar_mul(negmx[:], mx[:], -1.0)
            be = sm.tile([H, S], F32, tag="be")
            nc.scalar.activation(out=be[:], in_=band_sb[:], func=AF.Exp, bias=negmx[:])
            bsum = sm.tile([H, 1], F32, tag="bsum")
            nc.vector.reduce_sum(out=bsum[:], in_=be[:], axis=mybir.AxisListType.X)
            brcp = sm.tile([H, 1], F32, tag="brcp")
            nc.vector.reciprocal(out=brcp[:], in_=bsum[:])
            bw = sm.tile([H, S], F32, tag="bw")
            nc.vector.tensor_scalar_mul(bw[:], be[:], brcp[:])
            eps8 = sm.tile([H, 1], F32, tag="eps8")
            nc.vector.memset(eps8[:], 1e-8)
            bwsq = sm.tile([H, S], F32, tag="bwsq")
            nc.scalar.activation(out=bwsq[:], in_=bw[:], func=AF.Sqrt, bias=eps8[:])
            es2 = sm.tile([H, 1], F32, tag="es2")
            nc.scalar.activation(out=es2[:], in_=pbs_sb[:], func=AF.Exp, scale=0.5)
            bwsq_s = sm.tile([H, S], F32, tag="bwsq_s")
            nc.vector.tensor_scalar_mul(bwsq_s[:], bwsq[:], es2[:])
            bw2_d = dpool.tile([H, S], F32, tag="bw2_d")
            nc.gpsimd.dma_start(out=bw2_d[:], in_=bwsq_s[:])

            # mag_head = bw @ mag ; sqrtm = sqrt(mag_head) -> DRAM bounce
            mag_sb = big.tile([S, L], F32, tag="mag")
            nc.gpsimd.dma_start(out=mag_sb[:], in_=mag_d)
            mag_r = big.tile([S, L], BF16, tag="mag_r")
            nc.vector.tensor_copy(mag_r[:], mag_sb[:])
            bwT_ps = ps_mm.tile([128, 512], F32, tag="mm")
            nc.tensor.transpose(bwT_ps[0:S, 0:H], bw[:], ident[0:S, 0:H])
            bwT = sm.tile([S, H], BF16, tag="bwT")
            nc.vector.tensor_copy(bwT[:], bwT_ps[0:S, 0:H])
            sqrtm = big.tile([H, L], F32, tag="sqrtm")
            for jh in range(2):
                mg = ps_mm.tile([128, 512], F32, tag="mm")
                nc.tensor.matmul(mg[0:H, 0:512], bwT[:],
                                 mag_r[:, jh * 512:(jh + 1) * 512],
                                 start=True, stop=True)
                nc.scalar.activation(out=sqrtm[:, jh * 512:(jh + 1) * 512],
                                     in_=mg[0:H, 0:512], func=AF.Sqrt)
            sq_d = dpool.tile([H, L], F32, tag="sq_d")
            nc.gpsimd.dma_start(out=sq_d[:], in_=sqrtm[:])

            # feat tiles x=0..2: slot s (head h=3x+s) rows [32s, 32s+32):
            #   row 32s+15: mask[j], rows 32s+16..31: F, rest zero
            featKs = []
            for x in range(3):
                nx = NHX[x]
                fbase = big.tile([128, L], F32, tag=f"fbase{x}")
                nc.vector.memset(fbase[:], 0.0)
                smrep = big.tile([128, L], F32, tag=f"smrep{x}")
                nc.vector.memset(smrep[:], 0.0)
                fscale = big.tile([128, 1], F32, tag=f"fscale{x}")
                nc.vector.memset(fscale[:], 0.0)
                for s in range(nx):
                    h = 3 * x + s
                    nc.sync.dma_start(out=fbase[32 * s + 16:32 * s + 32, :],
                                      in_=cs_d)
                    row = sq_d[h:h + 1, :]
                    nc.sync.dma_start(out=smrep[32 * s + 16:32 * s + 32, :],
                                      in_=bass.AP(tensor=row.tensor,
                                                  offset=row.offset,
                                                  ap=[[0, 16]] + row.ap[1:]))
                    for cc in range(2):
                        nc.gpsimd.dma_start(
                            out=fscale[32 * s + 16 + 8 * cc:
                                       32 * s + 24 + 8 * cc, 0:1],
                            in_=bw2_d[h:h + 1, 0:S])
                featK = big.tile([128, L], BF16, tag=f"featK{x}")
                nc.vector.scalar_tensor_tensor(out=featK[:], in0=fbase[:],
                                               scalar=fscale[:, 0:1], in1=smrep[:],
                                               op0=OP.mult, op1=OP.mult)
                for s in range(nx):
                    nc.gpsimd.dma_start(
                        out=featK[32 * s + 15:32 * s + 16, :],
                        in_=mask_d[None, :])
                featKs.append(featK)

            # ============ inputs: hs, weights, biases =====================
            hs_all = big.tile([128, NJB * D], F32, tag="hs")      # (jb, d)
            nc.sync.dma_start(
                out=hs_all[:].rearrange("p (jb d) -> p jb d", d=D),
                in_=bass.AP(tensor=hs_d.tensor, offset=hs_d.offset,
                            ap=[[D, 128], [128 * D, NJB], [1, D]]))

            w_sb = {}
            for nm, wd in (("wq", wq_d), ("wk", wk_d), ("wv", wv_d)):
                traw = big.tile([128, 2 * D], F32, tag=nm + "_raw")  # (d1, dout)
                nc.sync.dma_start(
                    out=traw[:].rearrange("p (t d) -> p t d", d=D),
                    in_=bass.AP(tensor=wd.tensor, offset=wd.offset,
                                ap=[[D, 128], [128 * D, 2], [1, D]]))
                t = big.tile([128, 2 * D], BF16, tag=nm)
                nc.vector.tensor_copy(t[:], traw[:])
                w_sb[nm] = t
            wo_f = big.tile([128, 2 * D], F32, tag="wo_f")
            nc.sync.dma_start(
                out=wo_f[:].rearrange("p (t d) -> p t d", d=D),
                in_=bass.AP(tensor=wo_d.tensor, offset=wo_d.offset,
                            ap=[[D, 128], [128 * D, 2], [1, D]]))
            wo_bf = big.tile([128, 2 * D], BF16, tag="wo_bf")
            nc.vector.tensor_copy(wo_bf[:], wo_f[:])

            bq_sb = big.tile([32, H], F32, tag="bq")   # col h, row = dout%32
            nc.sync.dma_start(out=bq_sb[:], in_=bq_d.rearrange("(h p) -> p h", p=32))
            bk_sb = big.tile([32, H], F32, tag="bk")
            nc.sync.dma_start(out=bk_sb[:], in_=bk_d.rearrange("(h p) -> p h", p=32))

            bv_b = big.tile([128, D], F32, tag="bv_b")
            nc.sync.dma_start(out=bv_b[:], in_=bass.AP(
                tensor=bv_d.tensor, offset=bv_d.offset, ap=[[0, 128], [1, D]]))
            gamma_b = big.tile([128, D], F32, tag="gamma_b")
            nc.sync.dma_start(out=gamma_b[:], in_=bass.AP(
                tensor=gamma_d.tensor, offset=gamma_d.offset, ap=[[0, 128], [1, D]]))
            beta_b = big.tile([128, D], F32, tag="beta_b")
            nc.sync.dma_start(out=beta_b[:], in_=bass.AP(
                tensor=beta_d.tensor, offset=beta_d.offset, ap=[[0, 128], [1, D]]))
            bo_f = big.tile([1, D], F32, tag="bo_f")
            nc.sync.dma_start(out=bo_f[:], in_=bo_d[None, :])
            bo_bf = big.tile([1, D], BF16, tag="bo_bf")
            nc.vector.tensor_copy(bo_bf[:], bo_f[:])
            ones_bf = big.tile([1, 128], BF16, tag="ones_bf")
            nc.vector.memset(ones_bf[:], 1.0)

            # ============ hs^T (bf16) =====================================
            hsT = big.tile([128, 2 * L], BF16, tag="hsT")          # (d1, j)
            for jb in range(NJB):
                for d1 in range(2):
                    tp = ps_mm.tile([128, 512], F32, tag="mm")
                    nc.tensor.transpose(
                        tp[:, 0:128],
                        hs_all[:, jb * D + d1 * 128:jb * D + d1 * 128 + 128],
                        ident[:])
                    nc.vector.tensor_copy(
                        hsT[:, d1 * L + jb * 128:d1 * L + jb * 128 + 128],
                        tp[:, 0:128])

            # ============ V (+ones col) ===================================
            vaug = big.tile([128, NJB * H * 33], BF16, tag="vaug")
            vv = vaug[:].rearrange("p (jb h c) -> p jb h c", jb=NJB, h=H)
            nc.vector.memset(vv[:, :, :, 32:33], 1.0)
            for jb in range(NJB):
                ps = ps_mm.tile([128, 512], F32, tag="mm")
                for d1 in range(2):
                    nc.tensor.matmul(
                        ps[:, 0:D],
                        hsT[:, d1 * L + jb * 128:d1 * L + jb * 128 + 128],
                        w_sb["wv"][:, d1 * D:(d1 + 1) * D],
                        start=(d1 == 0), stop=(d1 == 1))
                nc.vector.tensor_add(vv[:, jb, :, 0:32],
                                     ps[:, 0:D].rearrange("p (h c) -> p h c", c=32),
                                     bv_b[:].rearrange("p (h c) -> p h c", c=32))

            # ============ attention, software-pipelined over heads ========
            ctx_all = big.tile([128, NIB * D], F32, tag="ctx")    # (ib, h, c)
            rcp_sb = big.tile([128, H * NIB], F32, tag="rcp")     # (h, ib)
            e_tiles = [None] * H
            kf_tiles = [None] * H
            qf_tiles = [None] * H

            def emit_kq(h):
                """q/k projections for head h + KF/QF assembly (K=49 fused)."""
                x, s = h // 3, h % 3
                kf = kfp.tile([64, L], BF16, tag="KF")
                qf = kfp.tile([64, LQ], BF16, tag="QF")
                kf_tiles[h], qf_tiles[h] = kf, qf
                ps = ps_mm.tile([128, 512], F32, tag="mm")
                for d1 in range(2):
                    nc.tensor.matmul(
                        ps[0:32, 0:LQ],
                        w_sb["wq"][:, d1 * D + 32 * h:d1 * D + 32 * h + 32],
                        hsT[:, d1 * L:d1 * L + LQ],
                        start=(d1 == 0), stop=(d1 == 1))
                nc.vector.tensor_scalar(qf[0:32, :], ps[0:32, 0:LQ],
                                        bq_sb[:, h:h + 1], QSCALE, OP.add, OP.mult)
                for jh in range(2):
                    ps = ps_mm.tile([128, 512], F32, tag="mm")
                    for d1 in range(2):
                        nc.tensor.matmul(
                            ps[0:32, 0:512],
                            w_sb["wk"][:, d1 * D + 32 * h:d1 * D + 32 * h + 32],
                            hsT[:, d1 * L + jh * 512:d1 * L + jh * 512 + 512],
                            start=(d1 == 0), stop=(d1 == 1))
                    nc.vector.tensor_scalar(kf[0:32, jh * 512:jh * 512 + 512],
                                            ps[0:32, 0:512], bk_sb[:, h:h + 1],
                                            None, OP.add)
                nc.gpsimd.dma_start(out=kf[32:49, :],
                                    in_=featKs[x][32 * s + 15:32 * s + 32, :])
                nc.vector.memset(qf[32:33, :], 1.0)
                nc.gpsimd.dma_start(out=qf[33:49, :],
                                    in_=featKs[x][32 * s + 16:32 * s + 32, 0:LQ])

            def emit_scores(h):
                eh = epool.tile([128, NJB * 512], BF16, tag="E")
                e_tiles[h] = eh
                kf, qf = kf_tiles[h], qf_tiles[h]
                for jp in range(NJB // 2):
                    sc = ps_sc.tile([128, 1024], F32, tag="sc")
                    for q in range(2):
                        jb = 2 * jp + q
                        nc.tensor.matmul(
                            sc[:, q * 512:(q + 1) * 512],
                            kf[0:49, jb * 128:jb * 128 + 128],
                            qf[0:49, :], start=True, stop=True)
                    nc.scalar.activation(out=eh[:, jp * 1024:(jp + 1) * 1024],
                                         in_=sc[:, 0:1024], func=AF.Exp)

            def emit_ctx(h):
                eh = e_tiles[h]
                ct = ps_ct.tile([128, NIB * 33], F32, tag="ct")
                for ib in range(NIB):
                    for jb in range(NJB):
                        nc.tensor.matmul(
                            ct[:, ib * 33:ib * 33 + 33],
                            eh[:, jb * 512 + ib * 128:jb * 512 + ib * 128 + 128],
                            vaug[:, (jb * H + h) * 33:(jb * H + h) * 33 + 33],
                            start=(jb == 0), stop=(jb == NJB - 1))
                ctr = ct[:].rearrange("p (ib c) -> p ib c", c=33)
                nc.vector.reciprocal(out=rcp_sb[:, h * NIB:(h + 1) * NIB],
                                     in_=ctr[:, :, 32:33])
                rc = rcp_sb[:, h * NIB:(h + 1) * NIB]
                rcb = bass.AP(tensor=rc.tensor, offset=rc.offset,
                              ap=[rc.ap[0], [rc.ap[1][0], NIB], [0, 32]])
                dst = ctx_all[:].rearrange("p (ib hh c) -> p ib hh c", ib=NIB, hh=H)
                nc.vector.tensor_tensor(out=dst[:, :, h, :], in0=ctr[:, :, 0:32],
                                        in1=rcb, op=OP.mult)

            emit_kq(0)
            emit_kq(1)
            for h in range(H):
                emit_scores(h)
                if h + 2 < H:
                    emit_kq(h + 2)
                if h > 0:
                    emit_ctx(h - 1)
            emit_ctx(H - 1)

            # ============ out-proj + residual + LayerNorm =================
            epsln = big.tile([128, 1], F32, tag="epsln")
            nc.vector.memset(epsln[:], LN_EPS)
            for ib in range(NIB):
                ctxT = sm.tile([128, 256], BF16, tag="ctxT")
                for t in range(2):
                    tp = ps_mm.tile([128, 512], F32, tag="mm")
                    nc.tensor.transpose(
                        tp[:, 0:128],
                        ctx_all[:, ib * D + t * 128:ib * D + t * 128 + 128],
                        ident[:])
                    nc.vector.tensor_copy(ctxT[:, t * 128:(t + 1) * 128], tp[:, 0:128])
                op_ps = ps_mm.tile([128, 512], F32, tag="mm")
                for t in range(2):
                    nc.tensor.matmul(op_ps[:, 0:D], ctxT[:, t * 128:(t + 1) * 128],
                                     wo_bf[:, t * D:(t + 1) * D],
                                     start=(t == 0), stop=False)
                nc.tensor.matmul(op_ps[:, 0:D], ones_bf[:], bo_bf[:],
                                 start=False, stop=True)

                x = sm.tile([128, D], F32, tag="x")
                nc.vector.tensor_add(x[:], op_ps[:, 0:D],
                                     hs_all[:, ib * D:(ib + 1) * D])
                stats = sm.tile([128, 6], F32, tag="stats")
                nc.vector.bn_stats(out=stats[:], in_=x[:])
                mv = sm.tile([128, 2], F32, tag="mv")
                nc.vector.bn_aggr(out=mv[:], in_=stats[:])
                std = sm.tile([128, 1], F32, tag="std")
                nc.scalar.activation(out=std[:], in_=mv[:, 1:2], func=AF.Sqrt,
                                     bias=epsln[:])
                rstd = sm.tile([128, 1], F32, tag="rstd")
                nc.vector.reciprocal(out=rstd[:], in_=std[:])
                xc = sm.tile([128, D], F32, tag="xc")
                nc.vector.tensor_scalar(xc[:], x[:], mv[:, 0:1], rstd[:],
                                        OP.subtract, OP.mult)
                y = sm.tile([128, D], F32, tag="y")
                nc.vector.tensor_mul(y[:], xc[:], gamma_b[:])
                y2 = sm.tile([128, D], F32, tag="y2")
                nc.vector.tensor_add(y2[:], y[:], beta_b[:])
                nc.sync.dma_start(out=out_d[ib * 128:(ib + 1) * 128, :], in_=y2[:])

    nc.compile()
    return nc


def kernel(**inputs):
    hs = np.asarray(inputs["hidden_states"], np.float32)
    am = np.asarray(inputs["attention_mask"], np.float32)
    cos = np.asarray(inputs["cos_phi"], np.float32)
    sin = np.asarray(inputs["sin_phi"], np.float32)
    mag = np.asarray(inputs["mag"], np.float32)

    if "nc" not in _CACHE:
        _CACHE["nc"] = _build()
    nc = _CACHE["nc"]

    in_maps = []
    for c in range(8):
        b, half = divmod(c, 2)
        r = half * LQ  # roll amount: odd cores see L rotated by -512

        def roll(x, axis):
            return np.roll(x, -r, axis=axis) if r else x

        m = {
            "hs": roll(hs[b], 0),
            "cs": roll(np.concatenate([cos[b], sin[b]], axis=0), 1),
            "mag": roll(mag[b], 1),
            "mask": roll(np.ascontiguousarray(
                np.broadcast_to(am[b, 0, 0], (L,))), 0),
            "wq": np.asarray(inputs["Wq"], np.float32),
            "wk": np.asarray(inputs["Wk"], np.float32),
            "wv": np.asarray(inputs["Wv"], np.float32),
            "wo": np.asarray(inputs["Wo"], np.float32),
            "bq": np.asarray(inputs["bq"], np.float32),
            "bk": np.asarray(inputs["bk"], np.float32),
            "bv": np.asarray(inputs["bv"], np.float32),
            "bo": np.asarray(inputs["bo"], np.float32),
            "band": np.asarray(inputs["band_logits"], np.float32),
            "pbs": np.asarray(inputs["phase_bias_scale"], np.float32),
            "gamma": np.asarray(inputs["ln_gamma"], np.float32),
            "beta": np.asarray(inputs["ln_beta"], np.float32),
        }
        in_maps.append(m)

    _CACHE["last_in_maps"] = in_maps
    globals()["_LAST_IN_MAPS"] = in_maps
    res = bass_utils.run_bass_kernel_spmd(nc, in_maps, core_ids=list(range(8)))
    out = np.empty((B, L, D), np.float32)
    for c in range(8):
        b, half = divmod(c, 2)
        out[b, half * LQ:(half + 1) * LQ, :] = res.results[c]["out"]
    return out


# revision 37
# speedup vs baseline: 1.6226x; 1.3574x over previous
"""PhaseSyncAttentionV4 Trainium2 Bass kernel.

Sharding: 8 cores = B(4) x query-halves(2). Core c handles batch b=c//2,
query rows [i0, i0+512), with full keys/values over L=1024. Everything is
core-local (LayerNorm is per-row) -> no collectives. SPMD: one program for
all cores; odd cores get their hs/cs/mag/mask rolled by -512 along L by
the host (attention is permutation-invariant over key positions), so the
query rows are always rows 0:512.

Math notes:
  - phase_scores * geo_mag * exp(pbs) folds into the QK matmul via
      geo = sqrt(m_i*m_j + 1e-8) ~= sqrt(m_i)*sqrt(m_j)
    (final output error ~1e-5, far below tolerance), giving per head a
    single score matmul with K=49 contraction rows:
      32 rows q/k (q pre-scaled by 1/sqrt(32) with bias)
      16 rows features F[f,t] = cs[f,t]*sqrt(bw_h[f]+1e-8)*e^{pbs_h/2}*sqrt(m_h[t])
       1 row  mask: KF carries mask[j], QF carries 1.0 -> contributes mask[j]
    where cs = [cos; sin] is the input's native [S, L] layout.
  - scores are computed transposed [j, i]: exp(E) is then used directly as
    the stationary matmul operand for both context and softmax row-sums
    (ones column appended to V), so no transposes of E are ever needed.
"""

import sys

sys.path.insert(0, "/opt/trn_rl_repo")

import math
import numpy as np

import concourse.bass as bass
import concourse.tile as tile
from concourse import bacc, mybir
from concourse import bass_utils
from concourse.masks import make_identity

F32 = mybir.dt.float32
BF16 = mybir.dt.bfloat16
AF = mybir.ActivationFunctionType
OP = mybir.AluOpType

B, L, D = 4, 1024, 256
H, S, DH = 8, 8, 32
LQ = 512          # queries per core
NJB = L // 128    # 8 key blocks
NIB = LQ // 128   # 4 query blocks
LN_EPS = 1e-12
QSCALE = 1.0 / math.sqrt(DH)
NHX = [3, 3, 2]   # heads per 96-row feature group

_CACHE = {}


def _build():
    nc = bacc.Bacc("TRN2", target_bir_lowering=False, debug=False,
                   enable_asserts=True, num_devices=8)

    dt_in = lambda n, s: nc.dram_tensor(n, s, F32, kind="ExternalInput").ap()
    bf_in = lambda n, s: nc.dram_tensor(n, s, BF16, kind="ExternalInput").ap()
    hs_d = dt_in("hs", [L, D])
    cs_d = bf_in("cs", [2 * S, L])
    mag_d = bf_in("mag", [S, L])
    mask_d = dt_in("mask", [L])
    wq_d, wk_d, wv_d, wo_d = (dt_in(n, [D, D]) for n in ("wq", "wk", "wv", "wo"))
    bq_d, bk_d, bv_d, bo_d = (dt_in(n, [D]) for n in ("bq", "bk", "bv", "bo"))
    bwrep_d = bf_in("bwrep", [S, 128])   # bw[h(p), r] at F-rows, else 0
    fsc2_d = dt_in("fsc2", [128])        # (bw[h,t]+1e-8)*e^{pbs_h} at F-rows
    gamma_d = dt_in("gamma", [D])
    beta_d = dt_in("beta", [D])
    out_d = nc.dram_tensor("out", [LQ, D], F32, kind="ExternalOutput").ap()

    with tile.TileContext(nc) as tc:
        with (
            tc.tile_pool(name="big", bufs=1) as big,
            tc.tile_pool(name="epool", bufs=2) as epool,
            tc.tile_pool(name="kf", bufs=3) as kfp,
            tc.tile_pool(name="sm", bufs=2) as sm,
            tc.tile_pool(name="dram", bufs=1, space="DRAM") as dpool,
            tc.tile_pool(name="ps_sc", bufs=2, space="PSUM") as ps_sc,
            tc.tile_pool(name="ps_ct", bufs=2, space="PSUM") as ps_ct,
            tc.tile_pool(name="ps_mm", bufs=2, space="PSUM") as ps_mm,
        ):
            ident = big.tile([128, 128], F32, tag="ident")
            make_identity(nc, ident[:])

            # ============ feature chain ===================================
            # smrep[p, j] = sqrt(mag_head[h(p), j]) * fscale[p] via one PE
            # matmul with a host-arranged replication matrix bwrep plus an
            # ACT sqrt with per-partition scale fsc2 = fscale^2:
            #   sqrt(mag_head * fsc2) = sqrt(mag_head) * fscale.
            # featK row 16h+f = F feature f of head h (dense, all 128 rows).
            mag_sb = big.tile([S, L], BF16, tag="mag")
            nc.gpsimd.dma_start(out=mag_sb[:], in_=mag_d)
            bwrep = big.tile([S, 128], BF16, tag="bwrep")
            nc.gpsimd.dma_start(out=bwrep[:], in_=bwrep_d)
            fsc2 = big.tile([128, 1], F32, tag="fsc2")
            nc.gpsimd.dma_start(out=fsc2[:], in_=fsc2_d[:, None])

            smrep = big.tile([128, L], BF16, tag="smrep")
            for jh in range(2):
                mg = ps_mm.tile([128, 512], F32, tag="mm")
                nc.tensor.matmul(mg[:, 0:512], bwrep[:],
                                 mag_sb[:, jh * 512:(jh + 1) * 512],
                                 start=True, stop=True)
                nc.scalar.activation(out=smrep[:, jh * 512:(jh + 1) * 512],
                                     in_=mg[:, 0:512], func=AF.Sqrt,
                                     scale=fsc2[:, 0:1])

            fbase = big.tile([128, L], BF16, tag="fbase")
            for h in range(H):
                nc.sync.dma_start(out=fbase[16 * h:16 * (h + 1), :], in_=cs_d)
            featK = big.tile([128, L], BF16, tag="featK")
            nc.vector.tensor_mul(featK[:], fbase[:], smrep[:])

            # ============ inputs: hs, weights, biases =====================
            hs_all = big.tile([128, NJB * D], F32, tag="hs")      # (jb, d)
            nc.sync.dma_start(
                out=hs_all[:].rearrange("p (jb d) -> p jb d", d=D),
                in_=bass.AP(tensor=hs_d.tensor, offset=hs_d.offset,
                            ap=[[D, 128], [128 * D, NJB], [1, D]]))

            w_sb = {}
            for nm, wd in (("wq", wq_d), ("wk", wk_d), ("wv", wv_d)):
                traw = big.tile([128, 2 * D], F32, tag=nm + "_raw")  # (d1, dout)
                nc.sync.dma_start(
                    out=traw[:].rearrange("p (t d) -> p t d", d=D),
                    in_=bass.AP(tensor=wd.tensor, offset=wd.offset,
                                ap=[[D, 128], [128 * D, 2], [1, D]]))
                t = big.tile([128, 2 * D], BF16, tag=nm)
                nc.vector.tensor_copy(t[:], traw[:])
                w_sb[nm] = t
            wo_f = big.tile([128, 2 * D], F32, tag="wo_f")
            nc.sync.dma_start(
                out=wo_f[:].rearrange("p (t d) -> p t d", d=D),
                in_=bass.AP(tensor=wo_d.tensor, offset=wo_d.offset,
                            ap=[[D, 128], [128 * D, 2], [1, D]]))
            wo_bf = big.tile([128, 2 * D], BF16, tag="wo_bf")
            nc.vector.tensor_copy(wo_bf[:], wo_f[:])

            bq_sb = big.tile([32, H], F32, tag="bq")   # col h, row = dout%32
            nc.sync.dma_start(out=bq_sb[:], in_=bq_d.rearrange("(h p) -> p h", p=32))
            bk_sb = big.tile([32, H], F32, tag="bk")
            nc.sync.dma_start(out=bk_sb[:], in_=bk_d.rearrange("(h p) -> p h", p=32))

            bv_b = big.tile([128, D], F32, tag="bv_b")
            nc.sync.dma_start(out=bv_b[:], in_=bass.AP(
                tensor=bv_d.tensor, offset=bv_d.offset, ap=[[0, 128], [1, D]]))
            gamma_b = big.tile([128, D], F32, tag="gamma_b")
            nc.sync.dma_start(out=gamma_b[:], in_=bass.AP(
                tensor=gamma_d.tensor, offset=gamma_d.offset, ap=[[0, 128], [1, D]]))
            beta_b = big.tile([128, D], F32, tag="beta_b")
            nc.sync.dma_start(out=beta_b[:], in_=bass.AP(
                tensor=beta_d.tensor, offset=beta_d.offset, ap=[[0, 128], [1, D]]))
            bo_f = big.tile([1, D], F32, tag="bo_f")
            nc.sync.dma_start(out=bo_f[:], in_=bo_d[None, :])
            bo_bf = big.tile([1, D], BF16, tag="bo_bf")
            nc.vector.tensor_copy(bo_bf[:], bo_f[:])
            ones_bf = big.tile([1, 128], BF16, tag="ones_bf")
            nc.vector.memset(ones_bf[:], 1.0)

            # ============ hs^T (bf16) =====================================
            hsT = big.tile([128, 2 * L], BF16, tag="hsT")          # (d1, j)
            for d1 in range(2):
                for jq in range(2):  # 4 consecutive jb per psum tile
                    tp = ps_mm.tile([128, 512], F32, tag="mm")
                    for u in range(4):
                        jb = 4 * jq + u
                        nc.tensor.transpose(
                            tp[:, u * 128:(u + 1) * 128],
                            hs_all[:, jb * D + d1 * 128:jb * D + d1 * 128 + 128],
                            ident[:])
                    nc.vector.tensor_copy(
                        hsT[:, d1 * L + jq * 512:d1 * L + jq * 512 + 512],
                        tp[:, 0:512])

            # ============ V (+ones col) ===================================
            vaug = big.tile([128, NJB * H * 33], BF16, tag="vaug")
            vv = vaug[:].rearrange("p (jb h c) -> p jb h c", jb=NJB, h=H)
            nc.vector.memset(vv[:, :, :, 32:33], 1.0)
            for jb in range(NJB):
                ps = ps_mm.tile([128, 512], F32, tag="mm")
                for d1 in range(2):
                    nc.tensor.matmul(
                        ps[:, 0:D],
                        hsT[:, d1 * L + jb * 128:d1 * L + jb * 128 + 128],
                        w_sb["wv"][:, d1 * D:(d1 + 1) * D],
                        start=(d1 == 0), stop=(d1 == 1))
                nc.vector.tensor_add(vv[:, jb, :, 0:32],
                                     ps[:, 0:D].rearrange("p (h c) -> p h c", c=32),
                                     bv_b[:].rearrange("p (h c) -> p h c", c=32))

            # ============ attention, software-pipelined over heads ========
            ctx_all = big.tile([128, NIB * D], F32, tag="ctx")    # (ib, h, c)
            rcp_sb = big.tile([128, H * NIB], F32, tag="rcp")     # (h, ib)
            e_tiles = [None] * H
            kf_tiles = [None] * H
            qf_tiles = [None] * H

            def emit_kq(h):
                """q/k projections for head h + KF/QF assembly (K=49 fused).
                Host pre-scales Wq/bq by 1/sqrt(DH)."""
                kf = kfp.tile([64, L], BF16, tag="KF")
                qf = kfp.tile([64, LQ], BF16, tag="QF")
                kf_tiles[h], qf_tiles[h] = kf, qf
                ps = ps_mm.tile([128, 512], F32, tag="mm")
                for d1 in range(2):
                    nc.tensor.matmul(
                        ps[0:32, 0:LQ],
                        w_sb["wq"][:, d1 * D + 32 * h:d1 * D + 32 * h + 32],
                        hsT[:, d1 * L:d1 * L + LQ],
                        start=(d1 == 0), stop=(d1 == 1))
                nc.vector.tensor_scalar_add(qf[0:32, :], ps[0:32, 0:LQ],
                                            bq_sb[:, h:h + 1])
                for jh in range(2):
                    ps = ps_mm.tile([128, 512], F32, tag="mm")
                    for d1 in range(2):
                        nc.tensor.matmul(
                            ps[0:32, 0:512],
                            w_sb["wk"][:, d1 * D + 32 * h:d1 * D + 32 * h + 32],
                            hsT[:, d1 * L + jh * 512:d1 * L + jh * 512 + 512],
                            start=(d1 == 0), stop=(d1 == 1))
                    nc.vector.tensor_scalar_add(kf[0:32, jh * 512:jh * 512 + 512],
                                                ps[0:32, 0:512], bk_sb[:, h:h + 1])
                nc.gpsimd.dma_start(out=kf[32:33, :], in_=mask_d[None, :])
                nc.gpsimd.dma_start(out=kf[33:49, :],
                                    in_=featK[16 * h:16 * h + 16, :])
                nc.vector.memset(qf[32:33, :], 1.0)
                nc.gpsimd.dma_start(out=qf[33:49, :],
                                    in_=featK[16 * h:16 * h + 16, 0:LQ])

            def emit_scores(h):
                eh = epool.tile([128, NJB * 512], BF16, tag="E")
                e_tiles[h] = eh
                kf, qf = kf_tiles[h], qf_tiles[h]
                for jp in range(NJB // 2):
                    sc = ps_sc.tile([128, 1024], F32, tag="sc")
                    for q in range(2):
                        jb = 2 * jp + q
                        nc.tensor.matmul(
                            sc[:, q * 512:(q + 1) * 512],
                            kf[0:49, jb * 128:jb * 128 + 128],
                            qf[0:49, :], start=True, stop=True)
                    nc.scalar.activation(out=eh[:, jp * 1024:(jp + 1) * 1024],
                                         in_=sc[:, 0:1024], func=AF.Exp)

            def emit_ctx(h):
                eh = e_tiles[h]
                ct = ps_ct.tile([128, NIB * 33], F32, tag="ct")
                for ib in range(NIB):
                    for jb in range(NJB):
                        nc.tensor.matmul(
                            ct[:, ib * 33:ib * 33 + 33],
                            eh[:, jb * 512 + ib * 128:jb * 512 + ib * 128 + 128],
                            vaug[:, (jb * H + h) * 33:(jb * H + h) * 33 + 33],
                            start=(jb == 0), stop=(jb == NJB - 1))
                ctr = ct[:].rearrange("p (ib c) -> p ib c", c=33)
                nc.vector.reciprocal(out=rcp_sb[:, h * NIB:(h + 1) * NIB],
                                     in_=ctr[:, :, 32:33])
                rc = rcp_sb[:, h * NIB:(h + 1) * NIB]
                rcb = bass.AP(tensor=rc.tensor, offset=rc.offset,
                              ap=[rc.ap[0], [rc.ap[1][0], NIB], [0, 32]])
                dst = ctx_all[:].rearrange("p (ib hh c) -> p ib hh c", ib=NIB, hh=H)
                nc.vector.tensor_tensor(out=dst[:, :, h, :], in0=ctr[:, :, 0:32],
                                        in1=rcb, op=OP.mult)

            emit_kq(0)
            emit_kq(1)
            for h in range(H):
                emit_scores(h)
                if h + 2 < H:
                    emit_kq(h + 2)
                if h > 0:
                    emit_ctx(h - 1)
            emit_ctx(H - 1)

            # ============ out-proj + residual + LayerNorm =================
            epsln = big.tile([128, 1], F32, tag="epsln")
            nc.vector.memset(epsln[:], LN_EPS)
            for ib in range(NIB):
                ctxT = sm.tile([128, 256], BF16, tag="ctxT")
                for t in range(2):
                    tp = ps_mm.tile([128, 512], F32, tag="mm")
                    nc.tensor.transpose(
                        tp[:, 0:128],
                        ctx_all[:, ib * D + t * 128:ib * D + t * 128 + 128],
                        ident[:])
                    nc.vector.tensor_copy(ctxT[:, t * 128:(t + 1) * 128], tp[:, 0:128])
                op_ps = ps_mm.tile([128, 512], F32, tag="mm")
                for t in range(2):
                    nc.tensor.matmul(op_ps[:, 0:D], ctxT[:, t * 128:(t + 1) * 128],
                                     wo_bf[:, t * D:(t + 1) * D],
                                     start=(t == 0), stop=False)
                nc.tensor.matmul(op_ps[:, 0:D], ones_bf[:], bo_bf[:],
                                 start=False, stop=True)

                x = sm.tile([128, D], F32, tag="x")
                nc.vector.tensor_add(x[:], op_ps[:, 0:D],
                                     hs_all[:, ib * D:(ib + 1) * D])
                stats = sm.tile([128, 6], F32, tag="stats")
                nc.vector.bn_stats(out=stats[:], in_=x[:])
                mv = sm.tile([128, 2], F32, tag="mv")
                nc.vector.bn_aggr(out=mv[:], in_=stats[:])
                std = sm.tile([128, 1], F32, tag="std")
                nc.scalar.activation(out=std[:], in_=mv[:, 1:2], func=AF.Sqrt,
                                     bias=epsln[:])
                rstd = sm.tile([128, 1], F32, tag="rstd")
                nc.vector.reciprocal(out=rstd[:], in_=std[:])
                xc = sm.tile([128, D], F32, tag="xc")
                nc.vector.tensor_scalar(xc[:], x[:], mv[:, 0:1], rstd[:],
                                        OP.subtract, OP.mult)
                y = sm.tile([128, D], F32, tag="y")
                nc.vector.tensor_mul(y[:], xc[:], gamma_b[:])
                y2 = sm.tile([128, D], F32, tag="y2")
                nc.vector.tensor_add(y2[:], y[:], beta_b[:])
                nc.sync.dma_start(out=out_d[ib * 128:(ib + 1) * 128, :], in_=y2[:])

    nc.compile()
    return nc


def make_in_maps(inputs):
    import ml_dtypes
    bf16 = ml_dtypes.bfloat16
    hs = np.asarray(inputs["hidden_states"], np.float32)
    am = np.asarray(inputs["attention_mask"], np.float32)
    cos = np.asarray(inputs["cos_phi"], np.float32)
    sin = np.asarray(inputs["sin_phi"], np.float32)
    mag = np.asarray(inputs["mag"], np.float32)

    # host-side constant prep (shape [8,8]/[8]-sized only): band softmax,
    # feature scale arrangement, and 1/sqrt(DH) folded into Wq/bq.
    band = np.asarray(inputs["band_logits"], np.float64)
    bwf = np.exp(band - band.max(axis=1, keepdims=True))
    bwf = (bwf / bwf.sum(axis=1, keepdims=True)).astype(np.float32)  # [H,S]
    pbs = np.asarray(inputs["phase_bias_scale"], np.float32)
    fsc2 = np.zeros(128, np.float32)       # row 16h+8c+t
    bwrep = np.zeros((S, 128), np.float32)  # [r, 16h+f] = bw[h, r]
    for h in range(H):
        for cc in range(2):
            fsc2[16 * h + 8 * cc:16 * h + 8 * cc + 8] =                 (bwf[h] + 1e-8) * np.exp(pbs[h])
        bwrep[:, 16 * h:16 * h + 16] = bwf[h][:, None]
    qs = np.float32(QSCALE)

    in_maps = []
    for c in range(8):
        b, half = divmod(c, 2)
        r = half * LQ  # roll amount: odd cores see L rotated by -512

        def roll(x, axis):
            return np.roll(x, -r, axis=axis) if r else x

        m = {
            "hs": roll(hs[b], 0),
            "cs": roll(np.concatenate([cos[b], sin[b]], axis=0), 1).astype(bf16),
            "mag": roll(mag[b], 1).astype(bf16),
            "mask": roll(np.ascontiguousarray(
                np.broadcast_to(am[b, 0, 0], (L,))), 0),
            "wq": np.asarray(inputs["Wq"], np.float32) * qs,
            "wk": np.asarray(inputs["Wk"], np.float32),
            "wv": np.asarray(inputs["Wv"], np.float32),
            "wo": np.asarray(inputs["Wo"], np.float32),
            "bq": np.asarray(inputs["bq"], np.float32) * qs,
            "bk": np.asarray(inputs["bk"], np.float32),
            "bv": np.asarray(inputs["bv"], np.float32),
            "bo": np.asarray(inputs["bo"], np.float32),
            "bwrep": bwrep.astype(bf16),
            "fsc2": fsc2,
            "gamma": np.asarray(inputs["ln_gamma"], np.float32),
            "beta": np.asarray(inputs["ln_beta"], np.float32),
        }
        in_maps.append(m)
    return in_maps


def kernel(**inputs):
    if "nc" not in _CACHE:
        _CACHE["nc"] = _build()
    nc = _CACHE["nc"]
    in_maps = make_in_maps(inputs)
    _CACHE["last_in_maps"] = in_maps
    globals()["_LAST_IN_MAPS"] = in_maps
    res = bass_utils.run_bass_kernel_spmd(nc, in_maps, core_ids=list(range(8)))
    out = np.empty((B, L, D), np.float32)
    for c in range(8):
        b, half = divmod(c, 2)
        out[b, half * LQ:(half + 1) * LQ, :] = res.results[c]["out"]
    return out


# revision 41
# speedup vs baseline: 1.6708x; 1.0297x over previous
"""PhaseSyncAttentionV4 Trainium2 Bass kernel.

Sharding: 8 cores = B(4) x query-halves(2). Core c handles batch b=c//2,
query rows [i0, i0+512), with full keys/values over L=1024. Everything is
core-local (LayerNorm is per-row) -> no collectives. SPMD: one program for
all cores; odd cores get their hs/cs/mag/mask rolled by -512 along L by
the host (attention is permutation-invariant over key positions), so the
query rows are always rows 0:512.

Math notes:
  - phase_scores * geo_mag * exp(pbs) folds into the QK matmul via
      geo = sqrt(m_i*m_j + 1e-8) ~= sqrt(m_i)*sqrt(m_j)
    (final output error ~1e-5, far below tolerance), giving per head a
    single score matmul with K=49 contraction rows:
      32 rows q/k (q pre-scaled by 1/sqrt(32) with bias)
      16 rows features F[f,t] = cs[f,t]*sqrt(bw_h[f]+1e-8)*e^{pbs_h/2}*sqrt(m_h[t])
       1 row  mask: KF carries mask[j], QF carries 1.0 -> contributes mask[j]
    where cs = [cos; sin] is the input's native [S, L] layout.
  - scores are computed transposed [j, i]: exp(E) is then used directly as
    the stationary matmul operand for both context and softmax row-sums
    (ones column appended to V), so no transposes of E are ever needed.
"""

import sys

sys.path.insert(0, "/opt/trn_rl_repo")

import math
import numpy as np

import concourse.bass as bass
import concourse.tile as tile
from concourse import bacc, mybir
from concourse import bass_utils
from concourse.masks import make_identity

F32 = mybir.dt.float32
BF16 = mybir.dt.bfloat16
AF = mybir.ActivationFunctionType
OP = mybir.AluOpType

B, L, D = 4, 1024, 256
H, S, DH = 8, 8, 32
LQ = 512          # queries per core
NJB = L // 128    # 8 key blocks
NIB = LQ // 128   # 4 query blocks
LN_EPS = 1e-12
QSCALE = 1.0 / math.sqrt(DH)
NHX = [3, 3, 2]   # heads per 96-row feature group

_CACHE = {}


def _build():
    nc = bacc.Bacc("TRN2", target_bir_lowering=False, debug=False,
                   enable_asserts=True, num_devices=8)

    dt_in = lambda n, s: nc.dram_tensor(n, s, F32, kind="ExternalInput").ap()
    bf_in = lambda n, s: nc.dram_tensor(n, s, BF16, kind="ExternalInput").ap()
    hs_d = dt_in("hs", [L, D])
    cs_d = bf_in("cs", [2 * S, L])
    mag_d = bf_in("mag", [S, L])
    mask_d = dt_in("mask", [L])
    wq_d, wk_d, wv_d, wo_d = (bf_in(n, [D, D]) for n in ("wq", "wk", "wv", "wo"))
    bq_d, bk_d, bv_d, bo_d = (dt_in(n, [D]) for n in ("bq", "bk", "bv", "bo"))
    bwrep_d = bf_in("bwrep", [S, 128])   # bw[h(p), r] at F-rows, else 0
    fsc2_d = dt_in("fsc2", [128])        # (bw[h,t]+1e-8)*e^{pbs_h} at F-rows
    gamma_d = dt_in("gamma", [D])
    beta_d = dt_in("beta", [D])
    out_d = nc.dram_tensor("out", [LQ, D], F32, kind="ExternalOutput").ap()

    with tile.TileContext(nc) as tc:
        with (
            tc.tile_pool(name="big", bufs=1) as big,
            tc.tile_pool(name="epool", bufs=2) as epool,
            tc.tile_pool(name="kf", bufs=3) as kfp,
            tc.tile_pool(name="sm", bufs=2) as sm,
            tc.tile_pool(name="dram", bufs=1, space="DRAM") as dpool,
            tc.tile_pool(name="ps_sc", bufs=2, space="PSUM") as ps_sc,
            tc.tile_pool(name="ps_ct", bufs=1, space="PSUM") as ps_ct,
            tc.tile_pool(name="ps_mm", bufs=2, space="PSUM") as ps_mm,
        ):
            ident = big.tile([128, 128], F32, tag="ident")
            make_identity(nc, ident[:])

            # ============ inputs: hs, weights, biases =====================
            hs_all = big.tile([128, NJB * D], F32, tag="hs")      # (jb, d)
            nc.sync.dma_start(
                out=hs_all[:].rearrange("p (jb d) -> p jb d", d=D),
                in_=bass.AP(tensor=hs_d.tensor, offset=hs_d.offset,
                            ap=[[D, 128], [128 * D, NJB], [1, D]]))

            w_sb = {}
            for nm, wd in (("wq", wq_d), ("wk", wk_d), ("wv", wv_d), ("wo", wo_d)):
                t = big.tile([128, 2 * D], BF16, tag=nm)
                nc.sync.dma_start(
                    out=t[:].rearrange("p (t d) -> p t d", d=D),
                    in_=bass.AP(tensor=wd.tensor, offset=wd.offset,
                                ap=[[D, 128], [128 * D, 2], [1, D]]))
                w_sb[nm] = t
            wo_bf = w_sb["wo"]

            bq_sb = big.tile([32, H], F32, tag="bq")   # col h, row = dout%32
            nc.sync.dma_start(out=bq_sb[:], in_=bq_d.rearrange("(h p) -> p h", p=32))
            bk_sb = big.tile([32, H], F32, tag="bk")
            nc.sync.dma_start(out=bk_sb[:], in_=bk_d.rearrange("(h p) -> p h", p=32))

            bv_b = big.tile([128, D], F32, tag="bv_b")
            nc.sync.dma_start(out=bv_b[:], in_=bass.AP(
                tensor=bv_d.tensor, offset=bv_d.offset, ap=[[0, 128], [1, D]]))
            gamma_b = big.tile([128, D], F32, tag="gamma_b")
            nc.sync.dma_start(out=gamma_b[:], in_=bass.AP(
                tensor=gamma_d.tensor, offset=gamma_d.offset, ap=[[0, 128], [1, D]]))
            beta_b = big.tile([128, D], F32, tag="beta_b")
            nc.sync.dma_start(out=beta_b[:], in_=bass.AP(
                tensor=beta_d.tensor, offset=beta_d.offset, ap=[[0, 128], [1, D]]))
            bo_f = big.tile([1, D], F32, tag="bo_f")
            nc.sync.dma_start(out=bo_f[:], in_=bo_d[None, :])
            bo_bf = big.tile([1, D], BF16, tag="bo_bf")
            nc.vector.tensor_copy(bo_bf[:], bo_f[:])
            ones_bf = big.tile([1, 128], BF16, tag="ones_bf")
            nc.vector.memset(ones_bf[:], 1.0)

            # ============ feature chain ===================================
            # smrep[p, j] = sqrt(mag_head[h(p), j]) * fscale[p] via one PE
            # matmul with a host-arranged replication matrix bwrep plus an
            # ACT sqrt with per-partition scale fsc2 = fscale^2:
            #   sqrt(mag_head * fsc2) = sqrt(mag_head) * fscale.
            # featK row 16h+f = F feature f of head h (dense, all 128 rows).
            mag_sb = big.tile([S, L], BF16, tag="mag")
            nc.gpsimd.dma_start(out=mag_sb[:], in_=mag_d)
            bwrep = big.tile([S, 128], BF16, tag="bwrep")
            nc.gpsimd.dma_start(out=bwrep[:], in_=bwrep_d)
            fsc2 = big.tile([128, 1], F32, tag="fsc2")
            nc.gpsimd.dma_start(out=fsc2[:], in_=fsc2_d[:, None])

            smrep = big.tile([128, L], BF16, tag="smrep")
            for jh in range(2):
                mg = ps_mm.tile([128, 512], F32, tag="mm")
                nc.tensor.matmul(mg[:, 0:512], bwrep[:],
                                 mag_sb[:, jh * 512:(jh + 1) * 512],
                                 start=True, stop=True)
                nc.scalar.activation(out=smrep[:, jh * 512:(jh + 1) * 512],
                                     in_=mg[:, 0:512], func=AF.Sqrt,
                                     scale=fsc2[:, 0:1])

            fbase = big.tile([128, L], BF16, tag="fbase")
            nc.sync.dma_start(out=fbase[:], in_=bass.AP(
                tensor=cs_d.tensor, offset=cs_d.offset,
                ap=[[0, H], [L, 2 * S], [1, L]]))
            featK = big.tile([128, L], BF16, tag="featK")
            nc.vector.tensor_mul(featK[:], fbase[:], smrep[:])

            # ============ hs^T (bf16) =====================================
            hsT = big.tile([128, 2 * L], BF16, tag="hsT")          # (d1, j)
            for d1 in range(2):
                for jq in range(2):  # 4 consecutive jb per psum tile
                    tp = ps_mm.tile([128, 512], F32, tag="mm")
                    for u in range(4):
                        jb = 4 * jq + u
                        nc.tensor.transpose(
                            tp[:, u * 128:(u + 1) * 128],
                            hs_all[:, jb * D + d1 * 128:jb * D + d1 * 128 + 128],
                            ident[:])
                    nc.vector.tensor_copy(
                        hsT[:, d1 * L + jq * 512:d1 * L + jq * 512 + 512],
                        tp[:, 0:512])

            # ============ V (+ones col) ===================================
            vaug = big.tile([128, NJB * H * 33], BF16, tag="vaug")
            vv = vaug[:].rearrange("p (jb h c) -> p jb h c", jb=NJB, h=H)
            nc.vector.memset(vv[:, :, :, 32:33], 1.0)
            for jb in range(NJB):
                ps = ps_mm.tile([128, 512], F32, tag="mm")
                for d1 in range(2):
                    nc.tensor.matmul(
                        ps[:, 0:D],
                        hsT[:, d1 * L + jb * 128:d1 * L + jb * 128 + 128],
                        w_sb["wv"][:, d1 * D:(d1 + 1) * D],
                        start=(d1 == 0), stop=(d1 == 1))
                nc.vector.tensor_add(vv[:, jb, :, 0:32],
                                     ps[:, 0:D].rearrange("p (h c) -> p h c", c=32),
                                     bv_b[:].rearrange("p (h c) -> p h c", c=32))

            # ============ attention, software-pipelined over heads ========
            ctx_all = big.tile([128, NIB * D], F32, tag="ctx")    # (ib, h, c)
            rcp_sb = big.tile([128, H * NIB], F32, tag="rcp")     # (h, ib)
            e_tiles = [None] * H
            kf_tiles = [None] * H
            qf_tiles = [None] * H

            def emit_kq(h):
                """q/k projections for head h + KF/QF assembly (K=49 fused).
                Host pre-scales Wq/bq by 1/sqrt(DH)."""
                kf = kfp.tile([64, L], BF16, tag="KF")
                qf = kfp.tile([64, LQ], BF16, tag="QF")
                kf_tiles[h], qf_tiles[h] = kf, qf
                ps = ps_mm.tile([128, 512], F32, tag="mm")
                for d1 in range(2):
                    nc.tensor.matmul(
                        ps[0:32, 0:LQ],
                        w_sb["wq"][:, d1 * D + 32 * h:d1 * D + 32 * h + 32],
                        hsT[:, d1 * L:d1 * L + LQ],
                        start=(d1 == 0), stop=(d1 == 1))
                nc.vector.tensor_scalar_add(qf[0:32, :], ps[0:32, 0:LQ],
                                            bq_sb[:, h:h + 1])
                for jh in range(2):
                    ps = ps_mm.tile([128, 512], F32, tag="mm")
                    for d1 in range(2):
                        nc.tensor.matmul(
                            ps[0:32, 0:512],
                            w_sb["wk"][:, d1 * D + 32 * h:d1 * D + 32 * h + 32],
                            hsT[:, d1 * L + jh * 512:d1 * L + jh * 512 + 512],
                            start=(d1 == 0), stop=(d1 == 1))
                    nc.vector.tensor_scalar_add(kf[0:32, jh * 512:jh * 512 + 512],
                                                ps[0:32, 0:512], bk_sb[:, h:h + 1])
                nc.gpsimd.dma_start(out=kf[32:33, :], in_=mask_d[None, :])
                nc.gpsimd.dma_start(out=kf[33:49, :],
                                    in_=featK[16 * h:16 * h + 16, :])
                nc.vector.memset(qf[32:33, :], 1.0)
                nc.gpsimd.dma_start(out=qf[33:49, :],
                                    in_=featK[16 * h:16 * h + 16, 0:LQ])

            def emit_scores(h):
                eh = epool.tile([128, NJB * 512], BF16, tag="E")
                e_tiles[h] = eh
                kf, qf = kf_tiles[h], qf_tiles[h]
                for jp in range(NJB // 2):
                    sc = ps_sc.tile([128, 1024], F32, tag="sc")
                    for q in range(2):
                        jb = 2 * jp + q
                        nc.tensor.matmul(
                            sc[:, q * 512:(q + 1) * 512],
                            kf[0:49, jb * 128:jb * 128 + 128],
                            qf[0:49, :], start=True, stop=True)
                    nc.scalar.activation(out=eh[:, jp * 1024:(jp + 1) * 1024],
                                         in_=sc[:, 0:1024], func=AF.Exp)

            def emit_ctx(h):
                eh = e_tiles[h]
                # two PSUM tiles (separate banks) so the 4 accumulation
                # chains don't all serialize on one bank
                ct0 = ps_ct.tile([128, 2 * 33], F32, tag="ct0")
                ct1 = ps_ct.tile([128, 2 * 33], F32, tag="ct1")
                cts = [ct0, ct1]
                for ph in range(2):      # chains ib=(ph, ph+2) run in parallel
                    for jb in range(NJB):
                        for u in range(2):
                            ib = 2 * u + ph
                            nc.tensor.matmul(
                                cts[u][:, ph * 33:ph * 33 + 33],
                                eh[:, jb * 512 + ib * 128:
                                    jb * 512 + ib * 128 + 128],
                                vaug[:, (jb * H + h) * 33:(jb * H + h) * 33 + 33],
                                start=(jb == 0), stop=(jb == NJB - 1))
                for u in range(2):
                    ctr = cts[u][:].rearrange("p (ib c) -> p ib c", c=33)
                    nc.vector.reciprocal(
                        out=rcp_sb[:, h * NIB + 2 * u:h * NIB + 2 * u + 2],
                        in_=ctr[:, :, 32:33])
                    rc = rcp_sb[:, h * NIB + 2 * u:h * NIB + 2 * u + 2]
                    rcb = bass.AP(tensor=rc.tensor, offset=rc.offset,
                                  ap=[rc.ap[0], [rc.ap[1][0], 2], [0, 32]])
                    dst = ctx_all[:].rearrange("p (ib hh c) -> p ib hh c",
                                               ib=NIB, hh=H)
                    nc.vector.tensor_tensor(out=dst[:, 2 * u:2 * u + 2, h, :],
                                            in0=ctr[:, :, 0:32],
                                            in1=rcb, op=OP.mult)

            emit_kq(0)
            emit_kq(1)
            for h in range(H):
                emit_scores(h)
                if h + 2 < H:
                    emit_kq(h + 2)
                if h > 0:
                    emit_ctx(h - 1)
            emit_ctx(H - 1)

            # ============ out-proj + residual + LayerNorm =================
            epsln = big.tile([128, 1], F32, tag="epsln")
            nc.vector.memset(epsln[:], LN_EPS)
            for ib in range(NIB):
                ctxT = sm.tile([128, 256], BF16, tag="ctxT")
                for t in range(2):
                    tp = ps_mm.tile([128, 512], F32, tag="mm")
                    nc.tensor.transpose(
                        tp[:, 0:128],
                        ctx_all[:, ib * D + t * 128:ib * D + t * 128 + 128],
                        ident[:])
                    nc.vector.tensor_copy(ctxT[:, t * 128:(t + 1) * 128], tp[:, 0:128])
                op_ps = ps_mm.tile([128, 512], F32, tag="mm")
                for t in range(2):
                    nc.tensor.matmul(op_ps[:, 0:D], ctxT[:, t * 128:(t + 1) * 128],
                                     wo_bf[:, t * D:(t + 1) * D],
                                     start=(t == 0), stop=False)
                nc.tensor.matmul(op_ps[:, 0:D], ones_bf[:], bo_bf[:],
                                 start=False, stop=True)

                x = sm.tile([128, D], F32, tag="x")
                nc.vector.tensor_add(x[:], op_ps[:, 0:D],
                                     hs_all[:, ib * D:(ib + 1) * D])
                stats = sm.tile([128, 6], F32, tag="stats")
                nc.vector.bn_stats(out=stats[:], in_=x[:])
                mv = sm.tile([128, 2], F32, tag="mv")
                nc.vector.bn_aggr(out=mv[:], in_=stats[:])
                std = sm.tile([128, 1], F32, tag="std")
                nc.scalar.activation(out=std[:], in_=mv[:, 1:2], func=AF.Sqrt,
                                     bias=epsln[:])
                rstd = sm.tile([128, 1], F32, tag="rstd")
                nc.vector.reciprocal(out=rstd[:], in_=std[:])
                xc = sm.tile([128, D], F32, tag="xc")
                nc.vector.tensor_scalar(xc[:], x[:], mv[:, 0:1], rstd[:],
                                        OP.subtract, OP.mult)
                y = sm.tile([128, D], F32, tag="y")
                nc.vector.tensor_mul(y[:], xc[:], gamma_b[:])
                y2 = sm.tile([128, D], F32, tag="y2")
                nc.vector.tensor_add(y2[:], y[:], beta_b[:])
                nc.sync.dma_start(out=out_d[ib * 128:(ib + 1) * 128, :], in_=y2[:])

    nc.compile()
    return nc


def make_in_maps(inputs):
    import ml_dtypes
    bf16 = ml_dtypes.bfloat16
    hs = np.asarray(inputs["hidden_states"], np.float32)
    am = np.asarray(inputs["attention_mask"], np.float32)
    cos = np.asarray(inputs["cos_phi"], np.float32)
    sin = np.asarray(inputs["sin_phi"], np.float32)
    mag = np.asarray(inputs["mag"], np.float32)

    # host-side constant prep (shape [8,8]/[8]-sized only): band softmax,
    # feature scale arrangement, and 1/sqrt(DH) folded into Wq/bq.
    band = np.asarray(inputs["band_logits"], np.float64)
    bwf = np.exp(band - band.max(axis=1, keepdims=True))
    bwf = (bwf / bwf.sum(axis=1, keepdims=True)).astype(np.float32)  # [H,S]
    pbs = np.asarray(inputs["phase_bias_scale"], np.float32)
    fsc2 = np.zeros(128, np.float32)       # row 16h+8c+t
    bwrep = np.zeros((S, 128), np.float32)  # [r, 16h+f] = bw[h, r]
    for h in range(H):
        for cc in range(2):
            fsc2[16 * h + 8 * cc:16 * h + 8 * cc + 8] =                 (bwf[h] + 1e-8) * np.exp(pbs[h])
        bwrep[:, 16 * h:16 * h + 16] = bwf[h][:, None]
    qs = np.float32(QSCALE)

    in_maps = []
    for c in range(8):
        b, half = divmod(c, 2)
        r = half * LQ  # roll amount: odd cores see L rotated by -512

        def roll(x, axis):
            return np.roll(x, -r, axis=axis) if r else x

        m = {
            "hs": roll(hs[b], 0),
            "cs": roll(np.concatenate([cos[b], sin[b]], axis=0), 1).astype(bf16),
            "mag": roll(mag[b], 1).astype(bf16),
            "mask": roll(np.ascontiguousarray(
                np.broadcast_to(am[b, 0, 0], (L,))), 0),
            "wq": (np.asarray(inputs["Wq"], np.float32) * qs).astype(bf16),
            "wk": np.asarray(inputs["Wk"], np.float32).astype(bf16),
            "wv": np.asarray(inputs["Wv"], np.float32).astype(bf16),
            "wo": np.asarray(inputs["Wo"], np.float32).astype(bf16),
            "bq": np.asarray(inputs["bq"], np.float32) * qs,
            "bk": np.asarray(inputs["bk"], np.float32),
            "bv": np.asarray(inputs["bv"], np.float32),
            "bo": np.asarray(inputs["bo"], np.float32),
            "bwrep": bwrep.astype(bf16),
            "fsc2": fsc2,
            "gamma": np.asarray(inputs["ln_gamma"], np.float32),
            "beta": np.asarray(inputs["ln_beta"], np.float32),
        }
        in_maps.append(m)
    return in_maps


def kernel(**inputs):
    if "nc" not in _CACHE:
        _CACHE["nc"] = _build()
    nc = _CACHE["nc"]
    in_maps = make_in_maps(inputs)
    _CACHE["last_in_maps"] = in_maps
    globals()["_LAST_IN_MAPS"] = in_maps
    res = bass_utils.run_bass_kernel_spmd(nc, in_maps, core_ids=list(range(8)))
    out = np.empty((B, L, D), np.float32)
    for c in range(8):
        b, half = divmod(c, 2)
        out[b, half * LQ:(half + 1) * LQ, :] = res.results[c]["out"]
    return out
